# revision 5
# baseline (speedup 1.0000x reference)
"""GAT (2-layer, PyG-style) on 8 Trainium2 NeuronCores via Bass/Tile.

Strategy (dst-major graph-parallel, v2):
  - Nodes globally sorted by in-degree and striped across cores (rank r ->
    core r%8, slot q=r//8).  Every core's block j (128 dsts) then has an
    near-identical degree profile, so one shared rectangular slot grid is
    tight (pad ~1.27x vs 1.48x before).
  - Table row of node = core*6272 + q; the same permutation orders the
    phase-A matmul (h = x @ [W | W@Asrc | W@Adst]), so per-block adst falls
    out of the same matmul (no separate xperm input / matmuls).
  - dma_gather indices are int16 (<32768), so THREE overlapping table
    windows A=[0,32768) B=[8704,41472) C=[17408,50176) are used; per-dst
    edges are split A/B/C by a greedy prefix/suffix fill, which shrinks the
    forced-window maxima that dominated the 2-window split.
  - Pad slots point at per-core poison rows (q=6271) whose asrc is set to
    -1e4 on device, so exp() underflows to exact 0 and no mask multiply or
    alpha normalization per edge is needed; softmax scale (1/den) is
    applied once per dst after the slot reduction.
  - Gathers are issued per GROUP of consecutive blocks (fewer SWDGE calls).
  - Layer 1 output (elu'd) returns to host, which reassembles/transposes
    and launches layer 2 (same machinery, 1 head, 16 classes).

kernel(**inputs) takes FULL unsharded inputs, returns the FULL [50000, 16]
output.  Host-side numpy does sharding/index prep only; all model math runs
on the NeuronCores.
"""

import os
import sys

import numpy as np

sys.path.insert(0, "/opt/trn_rl_repo")

import concourse.bacc as bacc
import concourse.bass as bass
import concourse.mybir as mybir
import concourse.tile as tile
from concourse.bass_utils import run_bass_kernel_spmd

F32 = mybir.dt.float32
BF16 = mybir.dt.bfloat16
I16 = mybir.dt.int16

N = 50000
NC = 8
FIN = 128
HID = 16
HEADS = 8
FH1 = HEADS * HID        # 128
CLS = 16
NEG = 0.2
NPAD = 50176             # 392 * 128
NSH = NPAD // NC         # 6272 rows per core shard
NSHC = NSH // 128        # 49 chunks per core == BLKS
BLKS = 49
OWNPAD = BLKS * 128      # 6272
OWN = N // NC            # 6250 real dsts per core
WIN = 32768              # int16 index window
OA, OB, OC = 0, 8704, NPAD - WIN          # 0, 8704, 17408
POISON_A = 6271                            # core0 q=6271 (< OB)
POISON_B = 2 * NSH + 6271 - OB             # core2 q=6271, B-window coords
POISON_C = 7 * NSH + 6271 - OC             # core7 q=6271 -> 32767

# Layer table layouts (f32-typed rows; gather moves bytes).
# L1 row (128 f32 = 512B): [h bf16 x128 (f32 cols 0:64) | psum-junk | asrc
# f32 x8 at cols 120:128].
ROW1 = 128
A1OFF = 120
# L2 row (64 f32 = 256B): [h2 bf16 x16 (f32 cols 0:8) | junk | asrc2 at 63]
ROW2 = 64
A2OFF = 63

GCAP1 = 72               # max slots per gather group, layer 1 (512B rows)
GCAP2 = 144              # layer 2 (256B rows)


# ---------------------------------------------------------------- host prep

def _prep(edge_index):
    """Degree-striped node layout + 3-window slot grid. Pure numpy."""
    ei = np.asarray(edge_index)
    loop = np.arange(N, dtype=np.int64)
    src = np.concatenate([ei[0].astype(np.int64), loop])
    dst = np.concatenate([ei[1].astype(np.int64), loop])

    deg_n = np.bincount(dst, minlength=N)
    order = np.argsort(-deg_n, kind="stable")      # rank r -> node
    rank = np.empty(N, np.int64)
    rank[order] = np.arange(N)
    core_n = rank % NC
    q_n = rank // NC                               # < 6250 <= 6271
    row_n = core_n * NSH + q_n                     # table/grid row

    dcore = core_n[dst]
    dq = q_n[dst]
    srow = row_n[src]
    key = dcore * NSH + dq                         # per-(core,dst) id
    M = NC * NSH
    blk_of = (np.arange(M) % NSH) // 128

    degq = np.bincount(key, minlength=M)
    kA = np.bincount(key[srow < OA + WIN], minlength=M)   # A-coverable prefix
    kC = np.bincount(key[srow >= OC], minlength=M)        # C-coverable suffix
    nA = np.bincount(key[srow < OB], minlength=M)         # A-only
    nC = np.bincount(key[srow >= OB + WIN], minlength=M)  # C-only

    def bmax(x):
        return x.reshape(NC, BLKS, 128).max(axis=(0, 2)).astype(np.int64)

    S_A = np.maximum(bmax(nA), 1)
    S_C = np.maximum(bmax(nC), 1)
    aA = np.minimum(kA, S_A[blk_of])
    remC = kC - np.maximum(0, aA - (degq - kC))
    aC = np.minimum(remC, S_C[blk_of])
    S_B = np.maximum(bmax(np.maximum(degq - aA - aC, 0)), 1)
    S_T = S_A + S_B + S_C

    ALOP = np.concatenate([[0], np.cumsum(S_A)]).astype(int)
    BLOP = np.concatenate([[0], np.cumsum(S_B)]).astype(int)
    CLOP = np.concatenate([[0], np.cumsum(S_C)]).astype(int)
    SLA, SLB, SLC = int(ALOP[-1]), int(BLOP[-1]), int(CLOP[-1])

    # per-edge slot assignment: sort by (dst-key, src-row); within dst the
    # first aA go to A, last aC to C, middle to B.
    eorder = np.lexsort((srow, key))
    ks, ss = key[eorder], srow[eorder]
    uniq, first_idx, counts = np.unique(ks, return_index=True,
                                        return_counts=True)
    pos = np.arange(len(ks)) - np.repeat(first_idx, counts)   # 0..deg-1
    deg_e = degq[ks]
    aA_e = aA[ks]
    aC_e = aC[ks]
    in_A = pos < aA_e
    in_C = pos >= (deg_e - aC_e)
    in_B = ~(in_A | in_C)
    blk_e = (ks % NSH) // 128
    p_e = (ks % NSH) % 128
    c_e = ks // NSH
    # feasibility / coverage asserts
    assert np.all(ss[in_A] < OA + WIN)
    assert np.all((ss[in_B] >= OB) & (ss[in_B] < OB + WIN))
    assert np.all(ss[in_C] >= OC)
    colA = ALOP[blk_e] + pos
    colB = BLOP[blk_e] + (pos - aA_e)
    colC = CLOP[blk_e] + (pos - (deg_e - aC_e))
    assert np.all(colA[in_A] < ALOP[blk_e[in_A] + 1])
    assert np.all(colB[in_B] < BLOP[blk_e[in_B] + 1])
    assert np.all(colC[in_C] < CLOP[blk_e[in_C] + 1])

    cores = []
    for c in range(NC):
        m = c_e == c
        idx_a = np.full((SLA, 128), POISON_A, np.int16)    # [col, partition]
        idx_b = np.full((SLB, 128), POISON_B, np.int16)
        idx_c = np.full((SLC, 128), POISON_C, np.int16)
        ma = m & in_A
        idx_a[colA[ma], p_e[ma]] = (ss[ma] - OA).astype(np.int16)
        mb = m & in_B
        idx_b[colB[mb], p_e[mb]] = (ss[mb] - OB).astype(np.int16)
        mc = m & in_C
        idx_c[colC[mc], p_e[mc]] = (ss[mc] - OC).astype(np.int16)
        cores.append(dict(
            idx_a=_wrap_idx(idx_a.T.copy()),
            idx_b=_wrap_idx(idx_b.T.copy()),
            idx_c=_wrap_idx(idx_c.T.copy()),
        ))

    # gather groups: consecutive blocks, slot total capped
    def mkgroups(cap):
        groups = []
        j = 0
        while j < BLKS:
            j2 = j + 1
            while j2 < BLKS and S_T[j:j2 + 1].sum() <= cap:
                j2 += 1
            groups.append((j, j2))
            j = j2
        return groups

    grids = dict(S_A=S_A, S_B=S_B, S_C=S_C, S_T=S_T,
                 ALOP=ALOP, BLOP=BLOP, CLOP=CLOP,
                 SLA=SLA, SLB=SLB, SLC=SLC,
                 groups1=mkgroups(GCAP1), groups2=mkgroups(GCAP2))
    # node ordering for host-side shard/unshard
    nodes_of_core = [order[c::NC] for c in range(NC)]      # index q -> node
    return cores, grids, nodes_of_core


def _wrap_idx(idx_pc):
    """[128, COLS] per-(partition,col) int16 -> dma_gather idx tile layout.

    dma_gather reads idx position i at sbuf [i%16, i//16] (int16), replicated
    across all 8 groups of 16 partitions.  Position i maps to output
    (partition i%128, col i//128).
    """
    P, C = idx_pc.shape
    assert P == 128
    flat = idx_pc.T.reshape(-1)            # position i = p + 128*c
    n16 = (len(flat) + 15) // 16
    t = np.zeros((16, n16), np.int16)
    t[np.arange(len(flat)) % 16, np.arange(len(flat)) // 16] = flat
    return np.tile(t, (8, 1))              # [128, n16]


# ------------------------------------------------------------- bass builder

def _build_layer(grids, layer):
    """One GAT layer as a Bass SPMD program.

    layer 1: FIN=128 in, 8 heads x 16 -> out 128 (elu'd)
    layer 2: 128 in, 1 head x 16 -> out 16 (+bias only)
    """
    S_A, S_B, S_C, S_T = (grids["S_A"], grids["S_B"], grids["S_C"],
                          grids["S_T"])
    ALOP, BLOP, CLOP = grids["ALOP"], grids["BLOP"], grids["CLOP"]
    SLA, SLB, SLC = grids["SLA"], grids["SLB"], grids["SLC"]
    groups = grids["groups1"] if layer == 1 else grids["groups2"]

    if layer == 1:
        FH, AH, ROW, AOFF = FH1, HEADS, ROW1, A1OFF
        WCOLS = FH + 2 * AH      # 144: [W | W@Asrc | W@Adst]
        FOUT = FH1
        CP0, CP1 = 72, 64        # tail copy psum[:, CP0:CP0+64] -> st[:, 64:]
    else:
        FH, AH, ROW, AOFF = CLS, 1, ROW2, A2OFF
        WCOLS = FH + 2 * AH      # 18
        FOUT = CLS

    nc = bacc.Bacc("TRN2", target_bir_lowering=False, debug=False,
                   num_devices=NC)
    xt = nc.declare_dram_parameter("xt", [128, NSH], BF16, isOutput=False)
    wext = nc.declare_dram_parameter("wext", [128, WCOLS], BF16,
                                     isOutput=False)
    brow = nc.declare_dram_parameter("brow", [128, FOUT], F32, isOutput=False)
    idxa = nc.declare_dram_parameter("idxa", [128, 8 * SLA], I16,
                                     isOutput=False)
    idxb = nc.declare_dram_parameter("idxb", [128, 8 * SLB], I16,
                                     isOutput=False)
    idxc = nc.declare_dram_parameter("idxc", [128, 8 * SLC], I16,
                                     isOutput=False)
    prow = nc.declare_dram_parameter("prow", [1, AH], F32, isOutput=False)
    out = nc.declare_dram_parameter("out", [OWNPAD, FOUT], F32, isOutput=True)
    th_sh = nc.dram_tensor("th_sh", [NSH, ROW], F32)
    th = nc.dram_tensor("th", [NPAD, ROW], F32, addr_space="Shared")

    SMAX = int(S_T.max())

    with tile.TileContext(nc) as tc:
        with (
            tc.tile_pool(name="const", bufs=1) as cpool,
            tc.tile_pool(name="xa", bufs=4) as xpool,
            tc.tile_pool(name="stage", bufs=4) as spool,
            tc.tile_pool(name="psA", bufs=2, space="PSUM") as psA,
            tc.tile_pool(name="ga", bufs=2) as gapool,
            tc.tile_pool(name="gb", bufs=2) as gbpool,
            tc.tile_pool(name="gc", bufs=2) as gcpool,
            tc.tile_pool(name="ep", bufs=2) as epool,
            tc.tile_pool(name="msg", bufs=2) as mpool,
            tc.tile_pool(name="fin", bufs=3) as fpool,
        ):
            # constants
            w_sb = cpool.tile([128, WCOLS], BF16)
            nc.sync.dma_start(w_sb[:], wext[:])
            b_sb = cpool.tile([128, FOUT], F32)
            nc.sync.dma_start(b_sb[:], brow[:])
            ia_sb = cpool.tile([128, 8 * SLA], I16)
            nc.sync.dma_start(ia_sb[:], idxa[:])
            ib_sb = cpool.tile([128, 8 * SLB], I16)
            nc.sync.dma_start(ib_sb[:], idxb[:])
            ic_sb = cpool.tile([128, 8 * SLC], I16)
            nc.sync.dma_start(ic_sb[:], idxc[:])
            adst_all = cpool.tile([128, BLKS * AH], F32)

            # ---- phase A: th[n] = [h(n) bf16 | junk | asrc(n) f32]
            for i in range(NSHC):
                xt_t = xpool.tile([128, 128], BF16)
                nc.sync.dma_start(xt_t[:], xt[:, i * 128:(i + 1) * 128])
                ph = psA.tile([128, WCOLS], F32)
                nc.tensor.matmul(ph[:], xt_t[:], w_sb[:], start=True,
                                 stop=True)
                st = spool.tile([128, ROW], F32)
                # h -> bf16 (cast on copy); tail cols f32 incl asrc
                nc.scalar.copy(st.bitcast(BF16)[:, 0:FH], ph[:, 0:FH])
                if layer == 1:
                    nc.vector.tensor_copy(st[:, CP1:ROW],
                                          ph[:, CP0:CP0 + (ROW - CP1)])
                else:
                    nc.vector.memset(st[:, FH // 2:AOFF], 0.0)
                    nc.vector.tensor_copy(st[:, AOFF:AOFF + 1],
                                          ph[:, FH:FH + 1])
                nc.vector.tensor_copy(adst_all[:, i * AH:(i + 1) * AH],
                                      ph[:, FH + AH:FH + 2 * AH])
                nc.sync.dma_start(th_sh[i * 128:(i + 1) * 128, :], st[:])
            # poison row q=6271: asrc -> -1e4 so exp underflows to exact 0.
            # Same sync queue as the st stores above -> FIFO-ordered after
            # the chunk-48 store; the barrier below orders it vs AllGather.
            nc.sync.dma_start(th_sh[NSH - 1:NSH, AOFF:AOFF + AH], prow[:])

            tc.strict_bb_all_engine_barrier()
            nc.gpsimd.collective_compute(
                "AllGather", mybir.AluOpType.bypass,
                replica_groups=[list(range(NC))],
                ins=[th_sh[:]], outs=[th[:]])
            tc.strict_bb_all_engine_barrier()

            # ---- phase B: per gather-group of consecutive 128-dst blocks
            for (j0, j1) in groups:
                SAg = int(ALOP[j1] - ALOP[j0])
                SBg = int(BLOP[j1] - BLOP[j0])
                SCg = int(CLOP[j1] - CLOP[j0])
                ga = gapool.tile([128, SAg, ROW], F32, tag="ga")
                nc.gpsimd.dma_gather(
                    ga[:], th[OA:OA + WIN, :],
                    ia_sb[:, 8 * ALOP[j0]: 8 * (ALOP[j0] + SAg)],
                    num_idxs=128 * SAg, num_idxs_reg=128 * SAg,
                    elem_size=ROW, single_packet=False)
                gb = gbpool.tile([128, SBg, ROW], F32, tag="gb")
                nc.gpsimd.dma_gather(
                    gb[:], th[OB:OB + WIN, :],
                    ib_sb[:, 8 * BLOP[j0]: 8 * (BLOP[j0] + SBg)],
                    num_idxs=128 * SBg, num_idxs_reg=128 * SBg,
                    elem_size=ROW, single_packet=False)
                gc = gcpool.tile([128, SCg, ROW], F32, tag="gc")
                nc.gpsimd.dma_gather(
                    gc[:], th[OC:OC + WIN, :],
                    ic_sb[:, 8 * CLOP[j0]: 8 * (CLOP[j0] + SCg)],
                    num_idxs=128 * SCg, num_idxs_reg=128 * SCg,
                    elem_size=ROW, single_packet=False)

                for j in range(j0, j1):
                    Sa, Sb, Sc = int(S_A[j]), int(S_B[j]), int(S_C[j])
                    S = Sa + Sb + Sc
                    a0 = int(ALOP[j] - ALOP[j0])
                    b0 = int(BLOP[j] - BLOP[j0])
                    c0 = int(CLOP[j] - CLOP[j0])
                    adst = adst_all[:, j * AH:(j + 1) * AH]

                    # e = asrc + adst per slot (3 window sub-ranges)
                    e = epool.tile([128, S, AH], F32, tag="e")
                    for (gt, off, S0, Ssub) in ((ga, a0, 0, Sa),
                                                (gb, b0, Sa, Sb),
                                                (gc, c0, Sa + Sb, Sc)):
                        nc.vector.tensor_tensor(
                            e[:, S0:S0 + Ssub, :],
                            gt[:, off:off + Ssub, AOFF:AOFF + AH],
                            adst.unsqueeze(1).broadcast_to([128, Ssub, AH]),
                            op=mybir.AluOpType.add)
                    # lrelu: max(NEG*e, e), then exp (ACT engine)
                    e2 = epool.tile([128, S, AH], F32, tag="e2")
                    nc.vector.scalar_tensor_tensor(
                        e2[:], e[:], NEG, e[:],
                        op0=mybir.AluOpType.mult, op1=mybir.AluOpType.max)
                    pt = epool.tile([128, S, AH], F32, tag="p")
                    nc.scalar.activation(pt[:], e2[:],
                                         mybir.ActivationFunctionType.Exp)

                    den = fpool.tile([128, AH], F32, tag="den")
                    nc.vector.tensor_reduce(den[:],
                                            pt[:].transpose([0, 2, 1]),
                                            axis=mybir.AxisListType.X,
                                            op=mybir.AluOpType.add)
                    rec = fpool.tile([128, AH], F32, tag="rec")
                    nc.vector.reciprocal(rec[:], den[:])

                    # msg = h_gath * p  (contiguous [p, s, fh], bf16)
                    msg = mpool.tile([128, S, FH], BF16, tag="msg")
                    for (gt, off, S0, Ssub) in ((ga, a0, 0, Sa),
                                                (gb, b0, Sa, Sb),
                                                (gc, c0, Sa + Sb, Sc)):
                        hview = gt.bitcast(BF16)[:, off:off + Ssub, 0:FH]
                        hview = hview.rearrange("p s (h c) -> p s h c", c=HID)
                        nc.vector.tensor_tensor(
                            msg[:, S0:S0 + Ssub]
                               .rearrange("p s (h c) -> p s h c", c=HID),
                            hview,
                            pt[:, S0:S0 + Ssub, :].unsqueeze(3)
                              .broadcast_to([128, Ssub, AH, HID]),
                            op=mybir.AluOpType.mult)
                    outun = fpool.tile([128, FOUT], F32, tag="outun")
                    nc.vector.tensor_reduce(outun[:],
                                            msg[:].transpose([0, 2, 1]),
                                            axis=mybir.AxisListType.X,
                                            op=mybir.AluOpType.add)

                    fin = fpool.tile([128, FOUT], F32, tag="fin")
                    if layer == 1:
                        # scale by 1/den (per head), +b, elu
                        nc.vector.tensor_tensor(
                            outun[:].rearrange("p (h c) -> p h c", c=HID),
                            outun[:].rearrange("p (h c) -> p h c", c=HID),
                            rec[:].unsqueeze(2).broadcast_to([128, AH, HID]),
                            op=mybir.AluOpType.mult)
                        nc.vector.tensor_tensor(outun[:], outun[:], b_sb[:],
                                                op=mybir.AluOpType.add)
                        # elu(z) = relu(z) + min(exp(z),1) - 1
                        a1 = fpool.tile([128, FOUT], F32, tag="a1")
                        nc.scalar.activation(a1[:], outun[:],
                                             mybir.ActivationFunctionType.Relu)
                        a2 = fpool.tile([128, FOUT], F32, tag="a2")
                        nc.scalar.activation(a2[:], outun[:],
                                             mybir.ActivationFunctionType.Exp)
                        nc.vector.scalar_tensor_tensor(
                            fin[:], a2[:], 1.0, a1[:],
                            op0=mybir.AluOpType.min, op1=mybir.AluOpType.add)
                        nc.vector.tensor_scalar_add(fin[:], fin[:], -1.0)
                    else:
                        # scale by 1/den (scalar per partition) on ACT, +b
                        sc = fpool.tile([128, FOUT], F32, tag="sc")
                        nc.scalar.activation(sc[:], outun[:],
                                             mybir.ActivationFunctionType.Copy,
                                             scale=rec[:])
                        nc.vector.tensor_tensor(fin[:], sc[:], b_sb[:],
                                                op=mybir.AluOpType.add)
                    nc.sync.dma_start(out[j * 128:(j + 1) * 128, :], fin[:])

    nc.compile()
    return nc


# --------------------------------------------------------------- execution

_CACHE = {}
TRACE = os.environ.get("GAT_TRACE", "0") == "1"
RUN_KW = {}


def _to_bf16(a):
    return np.asarray(a, np.float32).astype(mybir.dt.np(BF16))


def _amat(att, fh, hid, heads):
    """[heads, hid] attention vec -> [fh, heads] block-diag matrix."""
    m = np.zeros((fh, heads), np.float32)
    for h in range(heads):
        m[h * hid:(h + 1) * hid, h] = att[h]
    return m


def kernel(x, edge_index, W1, att_src1, att_dst1, b1, W2, att_src2, att_dst2,
           b2):
    x = np.asarray(x, np.float32)
    ei = np.asarray(edge_index)
    if "prep" not in _CACHE:
        _CACHE["prep"] = _prep(ei)
    cores, grids, nodes_of_core = _CACHE["prep"]

    if "nc1" not in _CACHE:
        _CACHE["nc1"] = _build_layer(grids, 1)
        _CACHE["nc2"] = _build_layer(grids, 2)
    nc1, nc2 = _CACHE["nc1"], _CACHE["nc2"]

    # ---- layer 1 inputs
    W1 = np.asarray(W1, np.float32)
    As1 = _amat(np.asarray(att_src1, np.float32), FH1, HID, HEADS)
    Ad1 = _amat(np.asarray(att_dst1, np.float32), FH1, HID, HEADS)
    w1ext = _to_bf16(np.concatenate([W1, W1 @ As1, W1 @ Ad1], axis=1))
    b1row = np.tile(np.asarray(b1, np.float32)[None, :], (128, 1))

    in_maps = []
    for c in range(NC):
        xsh = np.zeros((NSH, FIN), np.float32)
        xsh[:OWN] = x[nodes_of_core[c]]
        in_maps.append(dict(
            xt=_to_bf16(xsh.T.copy()),
            wext=w1ext, brow=b1row,
            prow=np.full((1, HEADS), -1e4, np.float32),
            idxa=cores[c]["idx_a"],
            idxb=cores[c]["idx_b"],
            idxc=cores[c]["idx_c"],
        ))
    res1 = run_bass_kernel_spmd(nc1, in_maps, list(range(NC)),
                                trace=TRACE, **RUN_KW)

    x2 = np.zeros((N, FH1), np.float32)
    for c in range(NC):
        x2[nodes_of_core[c]] = res1.results[c]["out"][:OWN]

    # ---- layer 2 inputs
    W2 = np.asarray(W2, np.float32)
    As2 = _amat(np.asarray(att_src2, np.float32), CLS, CLS, 1)
    Ad2 = _amat(np.asarray(att_dst2, np.float32), CLS, CLS, 1)
    w2ext = _to_bf16(np.concatenate([W2, W2 @ As2, W2 @ Ad2], axis=1))
    b2row = np.tile(np.asarray(b2, np.float32)[None, :], (128, 1))

    in_maps2 = []
    for c in range(NC):
        xsh = np.zeros((NSH, FH1), np.float32)
        xsh[:OWN] = x2[nodes_of_core[c]]
        in_maps2.append(dict(
            xt=_to_bf16(xsh.T.copy()),
            wext=w2ext, brow=b2row,
            prow=np.full((1, 1), -1e4, np.float32),
            idxa=in_maps[c]["idxa"],
            idxb=in_maps[c]["idxb"],
            idxc=in_maps[c]["idxc"],
        ))
    res2 = run_bass_kernel_spmd(nc2, in_maps2, list(range(NC)),
                                trace=TRACE, **RUN_KW)

    outf = np.zeros((N, CLS), np.float32)
    for c in range(NC):
        outf[nodes_of_core[c]] = res2.results[c]["out"][:OWN]
    kernel.last_results = (res1, res2)
    return outf


# revision 14
# speedup vs baseline: 1.0113x; 1.0113x over previous
"""GAT (2-layer, PyG-style) on 8 Trainium2 NeuronCores via Bass/Tile.

Strategy (dst-major graph-parallel, v2):
  - Nodes globally sorted by in-degree and striped across cores (rank r ->
    core r%8, slot q=r//8).  Every core's block j (128 dsts) then has an
    near-identical degree profile, so one shared rectangular slot grid is
    tight (pad ~1.27x vs 1.48x before).
  - Table row of node = core*6272 + q; the same permutation orders the
    phase-A matmul (h = x @ [W | W@Asrc | W@Adst]), so per-block adst falls
    out of the same matmul (no separate xperm input / matmuls).
  - dma_gather indices are int16 (<32768), so THREE overlapping table
    windows A=[0,32768) B=[8704,41472) C=[17408,50176) are used; per-dst
    edges are split A/B/C by a greedy prefix/suffix fill, which shrinks the
    forced-window maxima that dominated the 2-window split.
  - Pad slots point at per-core poison rows (q=6271) whose asrc is set to
    -1e4 on device, so exp() underflows to exact 0 and no mask multiply or
    alpha normalization per edge is needed; softmax scale (1/den) is
    applied once per dst after the slot reduction.
  - Gathers are issued per GROUP of consecutive blocks (fewer SWDGE calls).
  - Layer 1 output (elu'd) returns to host, which reassembles/transposes
    and launches layer 2 (same machinery, 1 head, 16 classes).

kernel(**inputs) takes FULL unsharded inputs, returns the FULL [50000, 16]
output.  Host-side numpy does sharding/index prep only; all model math runs
on the NeuronCores.
"""

import os
import sys

import numpy as np

sys.path.insert(0, "/opt/trn_rl_repo")

import concourse.bacc as bacc
import concourse.bass as bass
import concourse.mybir as mybir
import concourse.tile as tile
from concourse.bass_utils import run_bass_kernel_spmd

F32 = mybir.dt.float32
BF16 = mybir.dt.bfloat16
I16 = mybir.dt.int16

N = 50000
NC = 8
FIN = 128
HID = 16
HEADS = 8
FH1 = HEADS * HID        # 128
CLS = 16
NEG = 0.2
NPAD = 50176             # 392 * 128
NSH = NPAD // NC         # 6272 rows per core shard
NSHC = NSH // 128        # 49 chunks per core == BLKS
BLKS = 49
OWNPAD = BLKS * 128      # 6272
OWN = N // NC            # 6250 real dsts per core
WIN = 32768              # int16 index window
OA, OB, OC = 0, 8704, NPAD - WIN          # 0, 8704, 17408
POISON_A = 6271                            # core0 q=6271 (< OB)
POISON_B = 2 * NSH + 6271 - OB             # core2 q=6271, B-window coords
POISON_C = 7 * NSH + 6271 - OC             # core7 q=6271 -> 32767

# Layer table layouts (f32-typed rows; gather moves bytes).
# L1 row (128 f32 = 512B): [h bf16 x128 (f32 cols 0:64) | psum-junk | asrc
# f32 x8 at cols 120:128].
ROW1 = 128
A1OFF = 120
# L2 row (64 f32 = 256B): [h2 bf16 x16 (f32 cols 0:8) | junk | asrc2 at 63]
ROW2 = 64
A2OFF = 63

GCAP1 = 72               # max slots per gather group, layer 1 (512B rows)
GCAP2 = 144              # layer 2 (256B rows)


# ---------------------------------------------------------------- host prep

def _prep(edge_index):
    """Degree-striped node layout + 3-window slot grid. Pure numpy."""
    ei = np.asarray(edge_index)
    loop = np.arange(N, dtype=np.int64)
    src = np.concatenate([ei[0].astype(np.int64), loop])
    dst = np.concatenate([ei[1].astype(np.int64), loop])

    deg_n = np.bincount(dst, minlength=N)
    order = np.argsort(-deg_n, kind="stable")      # rank r -> node
    rank = np.empty(N, np.int64)
    rank[order] = np.arange(N)
    core_n = rank % NC
    q_n = rank // NC                               # < 6250 <= 6271
    row_n = core_n * NSH + q_n                     # table/grid row

    dcore = core_n[dst]
    dq = q_n[dst]
    srow = row_n[src]
    key = dcore * NSH + dq                         # per-(core,dst) id
    M = NC * NSH
    blk_of = (np.arange(M) % NSH) // 128

    degq = np.bincount(key, minlength=M)
    kA = np.bincount(key[srow < OA + WIN], minlength=M)   # A-coverable prefix
    kC = np.bincount(key[srow >= OC], minlength=M)        # C-coverable suffix
    nA = np.bincount(key[srow < OB], minlength=M)         # A-only
    nC = np.bincount(key[srow >= OB + WIN], minlength=M)  # C-only

    def bmax(x):
        return x.reshape(NC, BLKS, 128).max(axis=(0, 2)).astype(np.int64)

    # per-block caps: small search over bumps of the A/C caps to minimize
    # the induced B cap (greedy fill: A takes the sorted prefix, C the
    # suffix, B the middle).
    base_A = np.maximum(bmax(nA), 1)
    base_C = np.maximum(bmax(nC), 1)
    best_T = None
    for ba in range(3):
        for bc in range(3):
            SA = base_A + ba
            SC = base_C + bc
            aA_t = np.minimum(kA, SA[blk_of])
            remC_t = kC - np.maximum(0, aA_t - (degq - kC))
            aC_t = np.minimum(remC_t, SC[blk_of])
            SB = np.maximum(bmax(np.maximum(degq - aA_t - aC_t, 0)), 1)
            ST = SA + SB + SC
            if best_T is None:
                best_T = ST.copy()
                S_A, S_B, S_C = SA.copy(), SB.copy(), SC.copy()
            else:
                better = ST < best_T
                best_T = np.where(better, ST, best_T)
                S_A = np.where(better, SA, S_A)
                S_B = np.where(better, SB, S_B)
                S_C = np.where(better, SC, S_C)
    aA = np.minimum(kA, S_A[blk_of])
    remC = kC - np.maximum(0, aA - (degq - kC))
    aC = np.minimum(remC, S_C[blk_of])
    assert np.all(degq - aA - aC <= S_B[blk_of])
    S_T = S_A + S_B + S_C

    ALOP = np.concatenate([[0], np.cumsum(S_A)]).astype(int)
    BLOP = np.concatenate([[0], np.cumsum(S_B)]).astype(int)
    CLOP = np.concatenate([[0], np.cumsum(S_C)]).astype(int)
    SLA, SLB, SLC = int(ALOP[-1]), int(BLOP[-1]), int(CLOP[-1])

    # per-edge slot assignment: sort by (dst-key, src-row); within dst the
    # first aA go to A, last aC to C, middle to B.
    eorder = np.lexsort((srow, key))
    ks, ss = key[eorder], srow[eorder]
    uniq, first_idx, counts = np.unique(ks, return_index=True,
                                        return_counts=True)
    pos = np.arange(len(ks)) - np.repeat(first_idx, counts)   # 0..deg-1
    deg_e = degq[ks]
    aA_e = aA[ks]
    aC_e = aC[ks]
    in_A = pos < aA_e
    in_C = pos >= (deg_e - aC_e)
    in_B = ~(in_A | in_C)
    blk_e = (ks % NSH) // 128
    p_e = (ks % NSH) % 128
    c_e = ks // NSH
    # feasibility / coverage asserts
    assert np.all(ss[in_A] < OA + WIN)
    assert np.all((ss[in_B] >= OB) & (ss[in_B] < OB + WIN))
    assert np.all(ss[in_C] >= OC)
    colA = ALOP[blk_e] + pos
    colB = BLOP[blk_e] + (pos - aA_e)
    colC = CLOP[blk_e] + (pos - (deg_e - aC_e))
    assert np.all(colA[in_A] < ALOP[blk_e[in_A] + 1])
    assert np.all(colB[in_B] < BLOP[blk_e[in_B] + 1])
    assert np.all(colC[in_C] < CLOP[blk_e[in_C] + 1])

    cores = []
    for c in range(NC):
        m = c_e == c
        idx_a = np.full((SLA, 128), POISON_A, np.int16)    # [col, partition]
        idx_b = np.full((SLB, 128), POISON_B, np.int16)
        idx_c = np.full((SLC, 128), POISON_C, np.int16)
        ma = m & in_A
        idx_a[colA[ma], p_e[ma]] = (ss[ma] - OA).astype(np.int16)
        mb = m & in_B
        idx_b[colB[mb], p_e[mb]] = (ss[mb] - OB).astype(np.int16)
        mc = m & in_C
        idx_c[colC[mc], p_e[mc]] = (ss[mc] - OC).astype(np.int16)
        cores.append(dict(
            idx_a=_wrap_idx(idx_a.T.copy()),
            idx_b=_wrap_idx(idx_b.T.copy()),
            idx_c=_wrap_idx(idx_c.T.copy()),
        ))

    # gather groups: consecutive blocks, slot total capped
    def mkgroups(cap):
        groups = []
        j = 0
        while j < BLKS:
            j2 = j + 1
            while j2 < BLKS and S_T[j:j2 + 1].sum() <= cap:
                j2 += 1
            groups.append((j, j2))
            j = j2
        return groups

    grids = dict(S_A=S_A, S_B=S_B, S_C=S_C, S_T=S_T,
                 ALOP=ALOP, BLOP=BLOP, CLOP=CLOP,
                 SLA=SLA, SLB=SLB, SLC=SLC,
                 groups1=mkgroups(GCAP1), groups2=mkgroups(GCAP2))
    # node ordering for host-side shard/unshard
    nodes_of_core = [order[c::NC] for c in range(NC)]      # index q -> node
    return cores, grids, nodes_of_core


def _wrap_idx(idx_pc):
    """[128, COLS] per-(partition,col) int16 -> dma_gather idx tile layout.

    dma_gather reads idx position i at sbuf [i%16, i//16] (int16), replicated
    across all 8 groups of 16 partitions.  Position i maps to output
    (partition i%128, col i//128).
    """
    P, C = idx_pc.shape
    assert P == 128
    flat = idx_pc.T.reshape(-1)            # position i = p + 128*c
    n16 = (len(flat) + 15) // 16
    t = np.zeros((16, n16), np.int16)
    t[np.arange(len(flat)) % 16, np.arange(len(flat)) // 16] = flat
    return np.tile(t, (8, 1))              # [128, n16]


# ------------------------------------------------------------- bass builder

def _build_layer(grids, layer):
    """One GAT layer as a Bass SPMD program.

    layer 1: FIN=128 in, 8 heads x 16 -> out 128 (elu'd)
    layer 2: 128 in, 1 head x 16 -> out 16 (+bias only)
    """
    S_A, S_B, S_C, S_T = (grids["S_A"], grids["S_B"], grids["S_C"],
                          grids["S_T"])
    ALOP, BLOP, CLOP = grids["ALOP"], grids["BLOP"], grids["CLOP"]
    SLA, SLB, SLC = grids["SLA"], grids["SLB"], grids["SLC"]
    groups = grids["groups1"] if layer == 1 else grids["groups2"]

    if layer == 1:
        FH, AH, ROW, AOFF = FH1, HEADS, ROW1, A1OFF
        WCOLS = FH + 2 * AH      # 144: [W | W@Asrc | W@Adst]
        FOUT = FH1
        CP0, CP1 = 72, 64        # tail copy psum[:, CP0:CP0+64] -> st[:, 64:]
    else:
        FH, AH, ROW, AOFF = CLS, 1, ROW2, A2OFF
        WCOLS = FH + 2 * AH      # 18
        FOUT = CLS

    nc = bacc.Bacc("TRN2", target_bir_lowering=False, debug=False,
                   num_devices=NC)
    xt = nc.declare_dram_parameter("xt", [128, NSH], BF16, isOutput=False)
    wext = nc.declare_dram_parameter("wext", [128, WCOLS], BF16,
                                     isOutput=False)
    brow = nc.declare_dram_parameter("brow", [128, FOUT], F32, isOutput=False)
    idxa = nc.declare_dram_parameter("idxa", [128, 8 * SLA], I16,
                                     isOutput=False)
    idxb = nc.declare_dram_parameter("idxb", [128, 8 * SLB], I16,
                                     isOutput=False)
    idxc = nc.declare_dram_parameter("idxc", [128, 8 * SLC], I16,
                                     isOutput=False)
    prow = nc.declare_dram_parameter("prow", [1, AH], F32, isOutput=False)
    out = nc.declare_dram_parameter("out", [OWNPAD, FOUT], F32, isOutput=True)
    th_sh = nc.dram_tensor("th_sh", [NSH, ROW], F32)
    th = nc.dram_tensor("th", [NPAD, ROW], F32, addr_space="Shared")

    SMAX = int(S_T.max())

    with tile.TileContext(nc) as tc:
        with (
            tc.tile_pool(name="const", bufs=1) as cpool,
            tc.tile_pool(name="psA", bufs=2, space="PSUM") as psA,
            tc.tile_pool(name="ga", bufs=2) as gapool,
            tc.tile_pool(name="gb", bufs=2) as gbpool,
            tc.tile_pool(name="gc", bufs=2) as gcpool,
            tc.tile_pool(name="ep", bufs=2) as epool,
            tc.tile_pool(name="msg", bufs=2) as mpool,
            tc.tile_pool(name="fin", bufs=3) as fpool,
        ):
            # constants
            w_sb = cpool.tile([128, WCOLS], BF16)
            nc.sync.dma_start(w_sb[:], wext[:])
            b_sb = cpool.tile([128, FOUT], F32)
            nc.sync.dma_start(b_sb[:], brow[:])
            ia_sb = cpool.tile([128, 8 * SLA], I16)
            nc.sync.dma_start(ia_sb[:], idxa[:])
            ib_sb = cpool.tile([128, 8 * SLB], I16)
            nc.sync.dma_start(ib_sb[:], idxb[:])
            ic_sb = cpool.tile([128, 8 * SLC], I16)
            nc.sync.dma_start(ic_sb[:], idxc[:])
            adst_all = cpool.tile([128, BLKS * AH], F32)
            xt_sb = cpool.tile([128, NSH], BF16)
            nc.sync.dma_start(xt_sb[:], xt[:])
            st_all = cpool.tile([128, NSHC, ROW], F32)
            if layer == 2:
                nc.vector.memset(st_all[:], 0.0)

            stf = st_all[:].rearrange("p i r -> p (i r)")       # [128, NSHC*ROW]
            stb = st_all.bitcast(BF16)[:].rearrange("p i r -> p (i r)")

            # ---- phase A: th[n] = [h(n) bf16 | junk | asrc(n) f32]
            for i in range(NSHC):
                ph = psA.tile([128, WCOLS], F32)
                nc.tensor.matmul(ph[:], xt_sb[:, i * 128:(i + 1) * 128],
                                 w_sb[:], start=True, stop=True)
                # h -> bf16 (cast on copy); tail cols f32 incl asrc
                nc.scalar.copy(stb[:, i * 2 * ROW:i * 2 * ROW + FH],
                               ph[:, 0:FH])
                if layer == 1:
                    nc.vector.tensor_copy(
                        stf[:, i * ROW + CP1:(i + 1) * ROW],
                        ph[:, CP0:CP0 + (ROW - CP1)])
                else:
                    nc.vector.tensor_copy(
                        stf[:, i * ROW + AOFF:i * ROW + AOFF + 1],
                        ph[:, FH:FH + 1])
                nc.vector.tensor_copy(adst_all[:, i * AH:(i + 1) * AH],
                                      ph[:, FH + AH:FH + 2 * AH])
            nc.sync.dma_start(
                th_sh[:].rearrange("(i p) r -> p i r", p=128), st_all[:])
            # poison row q=6271: asrc -> -1e4 so exp underflows to exact 0.
            # Same sync queue as the store above -> FIFO-ordered after it;
            # the barrier below orders it vs AllGather.
            nc.sync.dma_start(th_sh[NSH - 1:NSH, AOFF:AOFF + AH], prow[:])

            tc.strict_bb_all_engine_barrier()
            nc.gpsimd.collective_compute(
                "AllGather", mybir.AluOpType.bypass,
                replica_groups=[list(range(NC))],
                ins=[th_sh[:]], outs=[th[:]])
            tc.strict_bb_all_engine_barrier()

            # ---- phase B: per gather-group of consecutive 128-dst blocks
            for (j0, j1) in groups:
                SAg = int(ALOP[j1] - ALOP[j0])
                SBg = int(BLOP[j1] - BLOP[j0])
                SCg = int(CLOP[j1] - CLOP[j0])
                ga = gapool.tile([128, SAg, ROW], F32, tag="ga")
                nc.gpsimd.dma_gather(
                    ga[:], th[OA:OA + WIN, :],
                    ia_sb[:, 8 * ALOP[j0]: 8 * (ALOP[j0] + SAg)],
                    num_idxs=128 * SAg, num_idxs_reg=128 * SAg,
                    elem_size=ROW, single_packet=False)
                gb = gbpool.tile([128, SBg, ROW], F32, tag="gb")
                nc.gpsimd.dma_gather(
                    gb[:], th[OB:OB + WIN, :],
                    ib_sb[:, 8 * BLOP[j0]: 8 * (BLOP[j0] + SBg)],
                    num_idxs=128 * SBg, num_idxs_reg=128 * SBg,
                    elem_size=ROW, single_packet=False)
                gc = gcpool.tile([128, SCg, ROW], F32, tag="gc")
                nc.gpsimd.dma_gather(
                    gc[:], th[OC:OC + WIN, :],
                    ic_sb[:, 8 * CLOP[j0]: 8 * (CLOP[j0] + SCg)],
                    num_idxs=128 * SCg, num_idxs_reg=128 * SCg,
                    elem_size=ROW, single_packet=False)

                nblk = j1 - j0
                fing = fpool.tile([128, nblk, FOUT], F32, tag="fing")
                fing_f = fing[:].rearrange("p j f -> p (j f)")
                for j in range(j0, j1):
                    Sa, Sb, Sc = int(S_A[j]), int(S_B[j]), int(S_C[j])
                    S = Sa + Sb + Sc
                    a0 = int(ALOP[j] - ALOP[j0])
                    b0 = int(BLOP[j] - BLOP[j0])
                    c0 = int(CLOP[j] - CLOP[j0])
                    jj = j - j0
                    adst = adst_all[:, j * AH:(j + 1) * AH]

                    # e = asrc + adst per slot (3 window sub-ranges)
                    e = epool.tile([128, S, AH], F32, tag="e")
                    for (gt, off, S0, Ssub) in ((ga, a0, 0, Sa),
                                                (gb, b0, Sa, Sb),
                                                (gc, c0, Sa + Sb, Sc)):
                        if AH == 1:
                            # adst is a per-partition scalar: add on ACT
                            nc.scalar.activation(
                                e[:, S0:S0 + Ssub, :],
                                gt[:, off:off + Ssub, AOFF:AOFF + AH],
                                mybir.ActivationFunctionType.Identity,
                                bias=adst)
                        else:
                            nc.vector.tensor_tensor(
                                e[:, S0:S0 + Ssub, :],
                                gt[:, off:off + Ssub, AOFF:AOFF + AH],
                                adst.unsqueeze(1)
                                    .broadcast_to([128, Ssub, AH]),
                                op=mybir.AluOpType.add)
                    # lrelu: max(NEG*e, e), then exp (ACT engine)
                    e2 = epool.tile([128, S, AH], F32, tag="e2")
                    nc.vector.scalar_tensor_tensor(
                        e2[:], e[:], NEG, e[:],
                        op0=mybir.AluOpType.mult, op1=mybir.AluOpType.max)
                    pt = epool.tile([128, S, AH], F32, tag="p")
                    den = fpool.tile([128, AH], F32, tag="den")
                    if AH == 1:
                        # denominator falls out of the exp on ACT
                        nc.scalar.activation(pt[:], e2[:],
                                             mybir.ActivationFunctionType.Exp,
                                             accum_out=den[:])
                    else:
                        nc.scalar.activation(pt[:], e2[:],
                                             mybir.ActivationFunctionType.Exp)
                        nc.vector.tensor_reduce(den[:],
                                                pt[:].transpose([0, 2, 1]),
                                                axis=mybir.AxisListType.X,
                                                op=mybir.AluOpType.add)
                    rec = fpool.tile([128, AH], F32, tag="rec")
                    nc.vector.reciprocal(rec[:], den[:])

                    # msg = h_gath * p  (contiguous [p, s, fh], bf16)
                    msg = mpool.tile([128, S, FH], BF16, tag="msg")
                    for (gt, off, S0, Ssub) in ((ga, a0, 0, Sa),
                                                (gb, b0, Sa, Sb),
                                                (gc, c0, Sa + Sb, Sc)):
                        hview = gt.bitcast(BF16)[:, off:off + Ssub, 0:FH]
                        hview = hview.rearrange("p s (h c) -> p s h c", c=HID)
                        nc.vector.tensor_tensor(
                            msg[:, S0:S0 + Ssub]
                               .rearrange("p s (h c) -> p s h c", c=HID),
                            hview,
                            pt[:, S0:S0 + Ssub, :].unsqueeze(3)
                              .broadcast_to([128, Ssub, AH, HID]),
                            op=mybir.AluOpType.mult)
                    outun = fpool.tile([128, FOUT], F32, tag="outun")
                    nc.vector.tensor_reduce(outun[:],
                                            msg[:].transpose([0, 2, 1]),
                                            axis=mybir.AxisListType.X,
                                            op=mybir.AluOpType.add)

                    fin = fing_f[:, jj * FOUT:(jj + 1) * FOUT]
                    if layer == 1:
                        # scale by 1/den (per head), +b, elu
                        nc.vector.tensor_tensor(
                            outun[:].rearrange("p (h c) -> p h c", c=HID),
                            outun[:].rearrange("p (h c) -> p h c", c=HID),
                            rec[:].unsqueeze(2).broadcast_to([128, AH, HID]),
                            op=mybir.AluOpType.mult)
                        nc.vector.tensor_tensor(outun[:], outun[:], b_sb[:],
                                                op=mybir.AluOpType.add)
                        # elu(z) = relu(z) + min(exp(z),1) - 1
                        a1 = fpool.tile([128, FOUT], F32, tag="a1")
                        nc.scalar.activation(a1[:], outun[:],
                                             mybir.ActivationFunctionType.Relu)
                        a2 = fpool.tile([128, FOUT], F32, tag="a2")
                        nc.scalar.activation(a2[:], outun[:],
                                             mybir.ActivationFunctionType.Exp)
                        t3 = fpool.tile([128, FOUT], F32, tag="t3")
                        nc.vector.scalar_tensor_tensor(
                            t3[:], a2[:], 1.0, a1[:],
                            op0=mybir.AluOpType.min, op1=mybir.AluOpType.add)
                        nc.scalar.activation(fin, t3[:],
                                             mybir.ActivationFunctionType.Copy,
                                             bias=-1.0)
                    else:
                        # scale by 1/den (scalar per partition) on ACT, +b
                        sc = fpool.tile([128, FOUT], F32, tag="sc")
                        nc.scalar.activation(sc[:], outun[:],
                                             mybir.ActivationFunctionType.Copy,
                                             scale=rec[:])
                        nc.vector.tensor_tensor(fin, sc[:], b_sb[:],
                                                op=mybir.AluOpType.add)
                nc.sync.dma_start(
                    out[j0 * 128:j1 * 128, :]
                    .rearrange("(jj p) f -> p jj f", p=128),
                    fing[:])

    nc.compile()
    return nc


# --------------------------------------------------------------- execution

_CACHE = {}
TRACE = os.environ.get("GAT_TRACE", "0") == "1"
RUN_KW = {}


def _to_bf16(a):
    return np.asarray(a, np.float32).astype(mybir.dt.np(BF16))


def _amat(att, fh, hid, heads):
    """[heads, hid] attention vec -> [fh, heads] block-diag matrix."""
    m = np.zeros((fh, heads), np.float32)
    for h in range(heads):
        m[h * hid:(h + 1) * hid, h] = att[h]
    return m


def kernel(x, edge_index, W1, att_src1, att_dst1, b1, W2, att_src2, att_dst2,
           b2):
    x = np.asarray(x, np.float32)
    ei = np.asarray(edge_index)
    if "prep" not in _CACHE:
        _CACHE["prep"] = _prep(ei)
    cores, grids, nodes_of_core = _CACHE["prep"]

    if "nc1" not in _CACHE:
        _CACHE["nc1"] = _build_layer(grids, 1)
        _CACHE["nc2"] = _build_layer(grids, 2)
    nc1, nc2 = _CACHE["nc1"], _CACHE["nc2"]

    # ---- layer 1 inputs
    W1 = np.asarray(W1, np.float32)
    As1 = _amat(np.asarray(att_src1, np.float32), FH1, HID, HEADS)
    Ad1 = _amat(np.asarray(att_dst1, np.float32), FH1, HID, HEADS)
    w1ext = _to_bf16(np.concatenate([W1, W1 @ As1, W1 @ Ad1], axis=1))
    b1row = np.tile(np.asarray(b1, np.float32)[None, :], (128, 1))

    in_maps = []
    for c in range(NC):
        xsh = np.zeros((NSH, FIN), np.float32)
        xsh[:OWN] = x[nodes_of_core[c]]
        in_maps.append(dict(
            xt=_to_bf16(xsh.T.copy()),
            wext=w1ext, brow=b1row,
            prow=np.full((1, HEADS), -1e4, np.float32),
            idxa=cores[c]["idx_a"],
            idxb=cores[c]["idx_b"],
            idxc=cores[c]["idx_c"],
        ))
    res1 = run_bass_kernel_spmd(nc1, in_maps, list(range(NC)),
                                trace=TRACE, **RUN_KW)

    x2 = np.zeros((N, FH1), np.float32)
    for c in range(NC):
        x2[nodes_of_core[c]] = res1.results[c]["out"][:OWN]

    # ---- layer 2 inputs
    W2 = np.asarray(W2, np.float32)
    As2 = _amat(np.asarray(att_src2, np.float32), CLS, CLS, 1)
    Ad2 = _amat(np.asarray(att_dst2, np.float32), CLS, CLS, 1)
    w2ext = _to_bf16(np.concatenate([W2, W2 @ As2, W2 @ Ad2], axis=1))
    b2row = np.tile(np.asarray(b2, np.float32)[None, :], (128, 1))

    in_maps2 = []
    for c in range(NC):
        xsh = np.zeros((NSH, FH1), np.float32)
        xsh[:OWN] = x2[nodes_of_core[c]]
        in_maps2.append(dict(
            xt=_to_bf16(xsh.T.copy()),
            wext=w2ext, brow=b2row,
            prow=np.full((1, 1), -1e4, np.float32),
            idxa=in_maps[c]["idxa"],
            idxb=in_maps[c]["idxb"],
            idxc=in_maps[c]["idxc"],
        ))
    res2 = run_bass_kernel_spmd(nc2, in_maps2, list(range(NC)),
                                trace=TRACE, **RUN_KW)

    outf = np.zeros((N, CLS), np.float32)
    for c in range(NC):
        outf[nodes_of_core[c]] = res2.results[c]["out"][:OWN]
    kernel.last_results = (res1, res2)
    return outf


# revision 16
# speedup vs baseline: 1.0810x; 1.0690x over previous
"""GAT (2-layer, PyG-style) on 8 Trainium2 NeuronCores via Bass/Tile.

Strategy (dst-major graph-parallel, v2):
  - Nodes globally sorted by in-degree and striped across cores (rank r ->
    core r%8, slot q=r//8).  Every core's block j (128 dsts) then has an
    near-identical degree profile, so one shared rectangular slot grid is
    tight (pad ~1.27x vs 1.48x before).
  - Table row of node = core*6272 + q; the same permutation orders the
    phase-A matmul (h = x @ [W | W@Asrc | W@Adst]), so per-block adst falls
    out of the same matmul (no separate xperm input / matmuls).
  - dma_gather indices are int16 (<32768), so THREE overlapping table
    windows A=[0,32768) B=[8704,41472) C=[17408,50176) are used; per-dst
    edges are split A/B/C by a greedy prefix/suffix fill, which shrinks the
    forced-window maxima that dominated the 2-window split.
  - Pad slots point at per-core poison rows (q=6271) whose asrc is set to
    -1e4 on device, so exp() underflows to exact 0 and no mask multiply or
    alpha normalization per edge is needed; softmax scale (1/den) is
    applied once per dst after the slot reduction.
  - Gathers are issued per GROUP of consecutive blocks (fewer SWDGE calls).
  - Layer 1 output (elu'd) returns to host, which reassembles/transposes
    and launches layer 2 (same machinery, 1 head, 16 classes).

kernel(**inputs) takes FULL unsharded inputs, returns the FULL [50000, 16]
output.  Host-side numpy does sharding/index prep only; all model math runs
on the NeuronCores.
"""

import os
import sys

import numpy as np

sys.path.insert(0, "/opt/trn_rl_repo")

import concourse.bacc as bacc
import concourse.bass as bass
import concourse.mybir as mybir
import concourse.tile as tile
from concourse.bass_utils import run_bass_kernel_spmd

F32 = mybir.dt.float32
BF16 = mybir.dt.bfloat16
I16 = mybir.dt.int16

N = 50000
NC = 8
FIN = 128
HID = 16
HEADS = 8
FH1 = HEADS * HID        # 128
CLS = 16
NEG = 0.2
NPAD = 50176             # 392 * 128
NSH = NPAD // NC         # 6272 rows per core shard
NSHC = NSH // 128        # 49 chunks per core == BLKS
BLKS = 49
OWNPAD = BLKS * 128      # 6272
OWN = N // NC            # 6250 real dsts per core
WIN = 32768              # int16 index window
OA, OB, OC = 0, 8704, NPAD - WIN          # 0, 8704, 17408
POISON_A = 6271                            # core0 q=6271 (< OB)
POISON_B = 2 * NSH + 6271 - OB             # core2 q=6271, B-window coords
POISON_C = 7 * NSH + 6271 - OC             # core7 q=6271 -> 32767

# Layer table layouts (f32-typed rows; gather moves bytes).
# L1 row (128 f32 = 512B): [h bf16 x128 (f32 cols 0:64) | psum-junk | asrc
# f32 x8 at cols 120:128].
ROW1 = 128
A1OFF = 120
# L2 row (64 f32 = 256B): [h2 bf16 x16 (f32 cols 0:8) | junk | asrc2 at 63]
ROW2 = 64
A2OFF = 63

GCAP1 = 72               # max slots per gather group, layer 1 (512B rows)
GCAP2 = 72               # layer 2 (256B rows)


# ---------------------------------------------------------------- host prep

def _prep(edge_index):
    """Degree-striped node layout + 3-window slot grid. Pure numpy."""
    ei = np.asarray(edge_index)
    loop = np.arange(N, dtype=np.int64)
    src = np.concatenate([ei[0].astype(np.int64), loop])
    dst = np.concatenate([ei[1].astype(np.int64), loop])

    deg_n = np.bincount(dst, minlength=N)
    order = np.argsort(-deg_n, kind="stable")      # rank r -> node
    rank = np.empty(N, np.int64)
    rank[order] = np.arange(N)
    core_n = rank % NC
    q_n = rank // NC                               # < 6250 <= 6271
    row_n = core_n * NSH + q_n                     # table/grid row

    dcore = core_n[dst]
    dq = q_n[dst]
    srow = row_n[src]
    key = dcore * NSH + dq                         # per-(core,dst) id
    M = NC * NSH
    blk_of = (np.arange(M) % NSH) // 128

    degq = np.bincount(key, minlength=M)
    kA = np.bincount(key[srow < OA + WIN], minlength=M)   # A-coverable prefix
    kC = np.bincount(key[srow >= OC], minlength=M)        # C-coverable suffix
    nA = np.bincount(key[srow < OB], minlength=M)         # A-only
    nC = np.bincount(key[srow >= OB + WIN], minlength=M)  # C-only

    def bmax(x):
        return x.reshape(NC, BLKS, 128).max(axis=(0, 2)).astype(np.int64)

    # per-block caps: small search over bumps of the A/C caps to minimize
    # the induced B cap (greedy fill: A takes the sorted prefix, C the
    # suffix, B the middle).
    base_A = np.maximum(bmax(nA), 1)
    base_C = np.maximum(bmax(nC), 1)
    best_T = None
    for ba in range(3):
        for bc in range(3):
            SA = base_A + ba
            SC = base_C + bc
            aA_t = np.minimum(kA, SA[blk_of])
            remC_t = kC - np.maximum(0, aA_t - (degq - kC))
            aC_t = np.minimum(remC_t, SC[blk_of])
            SB = np.maximum(bmax(np.maximum(degq - aA_t - aC_t, 0)), 1)
            ST = SA + SB + SC
            if best_T is None:
                best_T = ST.copy()
                S_A, S_B, S_C = SA.copy(), SB.copy(), SC.copy()
            else:
                better = ST < best_T
                best_T = np.where(better, ST, best_T)
                S_A = np.where(better, SA, S_A)
                S_B = np.where(better, SB, S_B)
                S_C = np.where(better, SC, S_C)
    aA = np.minimum(kA, S_A[blk_of])
    remC = kC - np.maximum(0, aA - (degq - kC))
    aC = np.minimum(remC, S_C[blk_of])
    assert np.all(degq - aA - aC <= S_B[blk_of])
    S_T = S_A + S_B + S_C

    ALOP = np.concatenate([[0], np.cumsum(S_A)]).astype(int)
    BLOP = np.concatenate([[0], np.cumsum(S_B)]).astype(int)
    CLOP = np.concatenate([[0], np.cumsum(S_C)]).astype(int)
    SLA, SLB, SLC = int(ALOP[-1]), int(BLOP[-1]), int(CLOP[-1])

    # per-edge slot assignment: sort by (dst-key, src-row); within dst the
    # first aA go to A, last aC to C, middle to B.
    eorder = np.lexsort((srow, key))
    ks, ss = key[eorder], srow[eorder]
    uniq, first_idx, counts = np.unique(ks, return_index=True,
                                        return_counts=True)
    pos = np.arange(len(ks)) - np.repeat(first_idx, counts)   # 0..deg-1
    deg_e = degq[ks]
    aA_e = aA[ks]
    aC_e = aC[ks]
    in_A = pos < aA_e
    in_C = pos >= (deg_e - aC_e)
    in_B = ~(in_A | in_C)
    blk_e = (ks % NSH) // 128
    p_e = (ks % NSH) % 128
    c_e = ks // NSH
    # feasibility / coverage asserts
    assert np.all(ss[in_A] < OA + WIN)
    assert np.all((ss[in_B] >= OB) & (ss[in_B] < OB + WIN))
    assert np.all(ss[in_C] >= OC)
    colA = ALOP[blk_e] + pos
    colB = BLOP[blk_e] + (pos - aA_e)
    colC = CLOP[blk_e] + (pos - (deg_e - aC_e))
    assert np.all(colA[in_A] < ALOP[blk_e[in_A] + 1])
    assert np.all(colB[in_B] < BLOP[blk_e[in_B] + 1])
    assert np.all(colC[in_C] < CLOP[blk_e[in_C] + 1])

    cores = []
    for c in range(NC):
        m = c_e == c
        idx_a = np.full((SLA, 128), POISON_A, np.int16)    # [col, partition]
        idx_b = np.full((SLB, 128), POISON_B, np.int16)
        idx_c = np.full((SLC, 128), POISON_C, np.int16)
        ma = m & in_A
        idx_a[colA[ma], p_e[ma]] = (ss[ma] - OA).astype(np.int16)
        mb = m & in_B
        idx_b[colB[mb], p_e[mb]] = (ss[mb] - OB).astype(np.int16)
        mc = m & in_C
        idx_c[colC[mc], p_e[mc]] = (ss[mc] - OC).astype(np.int16)
        cores.append(dict(
            idx_a=_wrap_idx(idx_a.T.copy()),
            idx_b=_wrap_idx(idx_b.T.copy()),
            idx_c=_wrap_idx(idx_c.T.copy()),
        ))

    # gather groups: consecutive blocks, slot total capped
    def mkgroups(cap):
        groups = []
        j = 0
        while j < BLKS:
            j2 = j + 1
            while j2 < BLKS and S_T[j:j2 + 1].sum() <= cap:
                j2 += 1
            groups.append((j, j2))
            j = j2
        return groups

    grids = dict(S_A=S_A, S_B=S_B, S_C=S_C, S_T=S_T,
                 ALOP=ALOP, BLOP=BLOP, CLOP=CLOP,
                 SLA=SLA, SLB=SLB, SLC=SLC,
                 groups1=mkgroups(GCAP1), groups2=mkgroups(GCAP2))
    # node ordering for host-side shard/unshard
    nodes_of_core = [order[c::NC] for c in range(NC)]      # index q -> node
    return cores, grids, nodes_of_core


def _wrap_idx(idx_pc):
    """[128, COLS] per-(partition,col) int16 -> dma_gather idx tile layout.

    dma_gather reads idx position i at sbuf [i%16, i//16] (int16), replicated
    across all 8 groups of 16 partitions.  Position i maps to output
    (partition i%128, col i//128).
    """
    P, C = idx_pc.shape
    assert P == 128
    flat = idx_pc.T.reshape(-1)            # position i = p + 128*c
    n16 = (len(flat) + 15) // 16
    t = np.zeros((16, n16), np.int16)
    t[np.arange(len(flat)) % 16, np.arange(len(flat)) // 16] = flat
    return np.tile(t, (8, 1))              # [128, n16]


# ------------------------------------------------------------- bass builder

def _build_layer(grids, layer):
    """One GAT layer as a Bass SPMD program.

    layer 1: FIN=128 in, 8 heads x 16 -> out 128 (elu'd)
    layer 2: 128 in, 1 head x 16 -> out 16 (+bias only)
    """
    S_A, S_B, S_C, S_T = (grids["S_A"], grids["S_B"], grids["S_C"],
                          grids["S_T"])
    ALOP, BLOP, CLOP = grids["ALOP"], grids["BLOP"], grids["CLOP"]
    SLA, SLB, SLC = grids["SLA"], grids["SLB"], grids["SLC"]
    groups = grids["groups1"] if layer == 1 else grids["groups2"]

    if layer == 1:
        FH, AH, ROW, AOFF = FH1, HEADS, ROW1, A1OFF
        WCOLS = FH + 2 * AH      # 144: [W | W@Asrc | W@Adst]
        FOUT = FH1
        CP0, CP1 = 72, 64        # tail copy psum[:, CP0:CP0+64] -> st[:, 64:]
    else:
        FH, AH, ROW, AOFF = CLS, 1, ROW2, A2OFF
        WCOLS = FH + 2 * AH      # 18
        FOUT = CLS

    nc = bacc.Bacc("TRN2", target_bir_lowering=False, debug=False,
                   num_devices=NC)
    xt = nc.declare_dram_parameter("xt", [128, NSH], BF16, isOutput=False)
    wext = nc.declare_dram_parameter("wext", [128, WCOLS], BF16,
                                     isOutput=False)
    brow = nc.declare_dram_parameter("brow", [128, FOUT], F32, isOutput=False)
    idxa = nc.declare_dram_parameter("idxa", [128, 8 * SLA], I16,
                                     isOutput=False)
    idxb = nc.declare_dram_parameter("idxb", [128, 8 * SLB], I16,
                                     isOutput=False)
    idxc = nc.declare_dram_parameter("idxc", [128, 8 * SLC], I16,
                                     isOutput=False)
    prow = nc.declare_dram_parameter("prow", [1, AH], F32, isOutput=False)
    out = nc.declare_dram_parameter("out", [OWNPAD, FOUT], F32, isOutput=True)
    th_sh = nc.dram_tensor("th_sh", [NSH, ROW], F32)
    th = nc.dram_tensor("th", [NPAD, ROW], F32, addr_space="Shared")

    SMAX = int(S_T.max())

    with tile.TileContext(nc) as tc:
        with (
            tc.tile_pool(name="const", bufs=1) as cpool,
            tc.tile_pool(name="psA", bufs=2, space="PSUM") as psA,
            tc.tile_pool(name="ga", bufs=2 if layer == 1 else 3) as gapool,
            tc.tile_pool(name="gb", bufs=2 if layer == 1 else 3) as gbpool,
            tc.tile_pool(name="gc", bufs=2 if layer == 1 else 3) as gcpool,
            tc.tile_pool(name="ep", bufs=2) as epool,
            tc.tile_pool(name="msg", bufs=2) as mpool,
            tc.tile_pool(name="fin", bufs=3) as fpool,
        ):
            # constants
            w_sb = cpool.tile([128, WCOLS], BF16)
            nc.sync.dma_start(w_sb[:], wext[:])
            b_sb = cpool.tile([128, FOUT], F32)
            nc.sync.dma_start(b_sb[:], brow[:])
            ia_sb = cpool.tile([128, 8 * SLA], I16)
            nc.sync.dma_start(ia_sb[:], idxa[:])
            ib_sb = cpool.tile([128, 8 * SLB], I16)
            nc.sync.dma_start(ib_sb[:], idxb[:])
            ic_sb = cpool.tile([128, 8 * SLC], I16)
            nc.sync.dma_start(ic_sb[:], idxc[:])
            adst_all = cpool.tile([128, BLKS * AH], F32)
            xt_sb = cpool.tile([128, NSH], BF16)
            nc.sync.dma_start(xt_sb[:], xt[:])
            st_all = cpool.tile([128, NSHC, ROW], F32)
            if layer == 2:
                nc.vector.memset(st_all[:], 0.0)

            stf = st_all[:].rearrange("p i r -> p (i r)")       # [128, NSHC*ROW]
            stb = st_all.bitcast(BF16)[:].rearrange("p i r -> p (i r)")

            # ---- phase A: th[n] = [h(n) bf16 | junk | asrc(n) f32]
            for i in range(NSHC):
                ph = psA.tile([128, WCOLS], F32)
                nc.tensor.matmul(ph[:], xt_sb[:, i * 128:(i + 1) * 128],
                                 w_sb[:], start=True, stop=True)
                # h -> bf16 (cast on copy); tail cols f32 incl asrc
                nc.scalar.copy(stb[:, i * 2 * ROW:i * 2 * ROW + FH],
                               ph[:, 0:FH])
                if layer == 1:
                    nc.vector.tensor_copy(
                        stf[:, i * ROW + CP1:(i + 1) * ROW],
                        ph[:, CP0:CP0 + (ROW - CP1)])
                else:
                    nc.vector.tensor_copy(
                        stf[:, i * ROW + AOFF:i * ROW + AOFF + 1],
                        ph[:, FH:FH + 1])
                nc.vector.tensor_copy(adst_all[:, i * AH:(i + 1) * AH],
                                      ph[:, FH + AH:FH + 2 * AH])
            nc.sync.dma_start(
                th_sh[:].rearrange("(i p) r -> p i r", p=128), st_all[:])
            # poison row q=6271: asrc -> -1e4 so exp underflows to exact 0.
            # Same sync queue as the store above -> FIFO-ordered after it;
            # the barrier below orders it vs AllGather.
            nc.sync.dma_start(th_sh[NSH - 1:NSH, AOFF:AOFF + AH], prow[:])

            tc.strict_bb_all_engine_barrier()
            nc.gpsimd.collective_compute(
                "AllGather", mybir.AluOpType.bypass,
                replica_groups=[list(range(NC))],
                ins=[th_sh[:]], outs=[th[:]])
            tc.strict_bb_all_engine_barrier()

            # ---- phase B: per gather-group of consecutive 128-dst blocks
            for (j0, j1) in groups:
                SAg = int(ALOP[j1] - ALOP[j0])
                SBg = int(BLOP[j1] - BLOP[j0])
                SCg = int(CLOP[j1] - CLOP[j0])
                ga = gapool.tile([128, SAg, ROW], F32, tag="ga")
                nc.gpsimd.dma_gather(
                    ga[:], th[OA:OA + WIN, :],
                    ia_sb[:, 8 * ALOP[j0]: 8 * (ALOP[j0] + SAg)],
                    num_idxs=128 * SAg, num_idxs_reg=128 * SAg,
                    elem_size=ROW, single_packet=False)
                gb = gbpool.tile([128, SBg, ROW], F32, tag="gb")
                nc.gpsimd.dma_gather(
                    gb[:], th[OB:OB + WIN, :],
                    ib_sb[:, 8 * BLOP[j0]: 8 * (BLOP[j0] + SBg)],
                    num_idxs=128 * SBg, num_idxs_reg=128 * SBg,
                    elem_size=ROW, single_packet=False)
                gc = gcpool.tile([128, SCg, ROW], F32, tag="gc")
                nc.gpsimd.dma_gather(
                    gc[:], th[OC:OC + WIN, :],
                    ic_sb[:, 8 * CLOP[j0]: 8 * (CLOP[j0] + SCg)],
                    num_idxs=128 * SCg, num_idxs_reg=128 * SCg,
                    elem_size=ROW, single_packet=False)

                nblk = j1 - j0
                fing = fpool.tile([128, nblk, FOUT], F32, tag="fing")
                fing_f = fing[:].rearrange("p j f -> p (j f)")
                for j in range(j0, j1):
                    Sa, Sb, Sc = int(S_A[j]), int(S_B[j]), int(S_C[j])
                    S = Sa + Sb + Sc
                    a0 = int(ALOP[j] - ALOP[j0])
                    b0 = int(BLOP[j] - BLOP[j0])
                    c0 = int(CLOP[j] - CLOP[j0])
                    jj = j - j0
                    adst = adst_all[:, j * AH:(j + 1) * AH]

                    # e = asrc + adst per slot (3 window sub-ranges)
                    e = epool.tile([128, S, AH], F32, tag="e")
                    for (gt, off, S0, Ssub) in ((ga, a0, 0, Sa),
                                                (gb, b0, Sa, Sb),
                                                (gc, c0, Sa + Sb, Sc)):
                        if AH == 1:
                            # adst is a per-partition scalar: add on ACT
                            nc.scalar.activation(
                                e[:, S0:S0 + Ssub, :],
                                gt[:, off:off + Ssub, AOFF:AOFF + AH],
                                mybir.ActivationFunctionType.Identity,
                                bias=adst)
                        else:
                            nc.vector.tensor_tensor(
                                e[:, S0:S0 + Ssub, :],
                                gt[:, off:off + Ssub, AOFF:AOFF + AH],
                                adst.unsqueeze(1)
                                    .broadcast_to([128, Ssub, AH]),
                                op=mybir.AluOpType.add)
                    # lrelu: max(NEG*e, e), then exp (ACT engine)
                    e2 = epool.tile([128, S, AH], F32, tag="e2")
                    nc.vector.scalar_tensor_tensor(
                        e2[:], e[:], NEG, e[:],
                        op0=mybir.AluOpType.mult, op1=mybir.AluOpType.max)
                    pt = epool.tile([128, S, AH], F32, tag="p")
                    den = fpool.tile([128, AH], F32, tag="den")
                    if AH == 1:
                        # denominator falls out of the exp on ACT
                        nc.scalar.activation(pt[:], e2[:],
                                             mybir.ActivationFunctionType.Exp,
                                             accum_out=den[:])
                    else:
                        nc.scalar.activation(pt[:], e2[:],
                                             mybir.ActivationFunctionType.Exp)
                        nc.vector.tensor_reduce(den[:],
                                                pt[:].transpose([0, 2, 1]),
                                                axis=mybir.AxisListType.X,
                                                op=mybir.AluOpType.add)
                    rec = fpool.tile([128, AH], F32, tag="rec")
                    nc.vector.reciprocal(rec[:], den[:])

                    # msg = h_gath * p  (contiguous [p, s, fh], bf16)
                    msg = mpool.tile([128, S, FH], BF16, tag="msg")
                    for (gt, off, S0, Ssub) in ((ga, a0, 0, Sa),
                                                (gb, b0, Sa, Sb),
                                                (gc, c0, Sa + Sb, Sc)):
                        hview = gt.bitcast(BF16)[:, off:off + Ssub, 0:FH]
                        hview = hview.rearrange("p s (h c) -> p s h c", c=HID)
                        nc.vector.tensor_tensor(
                            msg[:, S0:S0 + Ssub]
                               .rearrange("p s (h c) -> p s h c", c=HID),
                            hview,
                            pt[:, S0:S0 + Ssub, :].unsqueeze(3)
                              .broadcast_to([128, Ssub, AH, HID]),
                            op=mybir.AluOpType.mult)
                    outun = fpool.tile([128, FOUT], F32, tag="outun")
                    nc.vector.tensor_reduce(outun[:],
                                            msg[:].transpose([0, 2, 1]),
                                            axis=mybir.AxisListType.X,
                                            op=mybir.AluOpType.add)

                    fin = fing_f[:, jj * FOUT:(jj + 1) * FOUT]
                    if layer == 1:
                        # scale by 1/den (per head), +b, elu
                        nc.vector.tensor_tensor(
                            outun[:].rearrange("p (h c) -> p h c", c=HID),
                            outun[:].rearrange("p (h c) -> p h c", c=HID),
                            rec[:].unsqueeze(2).broadcast_to([128, AH, HID]),
                            op=mybir.AluOpType.mult)
                        nc.vector.tensor_tensor(outun[:], outun[:], b_sb[:],
                                                op=mybir.AluOpType.add)
                        # elu(z) = relu(z) + min(exp(z),1) - 1
                        a1 = fpool.tile([128, FOUT], F32, tag="a1")
                        nc.scalar.activation(a1[:], outun[:],
                                             mybir.ActivationFunctionType.Relu)
                        a2 = fpool.tile([128, FOUT], F32, tag="a2")
                        nc.scalar.activation(a2[:], outun[:],
                                             mybir.ActivationFunctionType.Exp)
                        t3 = fpool.tile([128, FOUT], F32, tag="t3")
                        nc.vector.scalar_tensor_tensor(
                            t3[:], a2[:], 1.0, a1[:],
                            op0=mybir.AluOpType.min, op1=mybir.AluOpType.add)
                        nc.scalar.activation(fin, t3[:],
                                             mybir.ActivationFunctionType.Copy,
                                             bias=-1.0)
                    else:
                        # scale by 1/den (scalar per partition) on ACT, +b
                        sc = fpool.tile([128, FOUT], F32, tag="sc")
                        nc.scalar.activation(sc[:], outun[:],
                                             mybir.ActivationFunctionType.Copy,
                                             scale=rec[:])
                        nc.vector.tensor_tensor(fin, sc[:], b_sb[:],
                                                op=mybir.AluOpType.add)
                nc.sync.dma_start(
                    out[j0 * 128:j1 * 128, :]
                    .rearrange("(jj p) f -> p jj f", p=128),
                    fing[:])

    nc.compile()
    return nc


# --------------------------------------------------------------- execution

_CACHE = {}
TRACE = os.environ.get("GAT_TRACE", "0") == "1"
RUN_KW = {}


def _to_bf16(a):
    return np.asarray(a, np.float32).astype(mybir.dt.np(BF16))


def _amat(att, fh, hid, heads):
    """[heads, hid] attention vec -> [fh, heads] block-diag matrix."""
    m = np.zeros((fh, heads), np.float32)
    for h in range(heads):
        m[h * hid:(h + 1) * hid, h] = att[h]
    return m


def kernel(x, edge_index, W1, att_src1, att_dst1, b1, W2, att_src2, att_dst2,
           b2):
    x = np.asarray(x, np.float32)
    ei = np.asarray(edge_index)
    if "prep" not in _CACHE:
        _CACHE["prep"] = _prep(ei)
    cores, grids, nodes_of_core = _CACHE["prep"]

    if "nc1" not in _CACHE:
        _CACHE["nc1"] = _build_layer(grids, 1)
        _CACHE["nc2"] = _build_layer(grids, 2)
    nc1, nc2 = _CACHE["nc1"], _CACHE["nc2"]

    # ---- layer 1 inputs
    W1 = np.asarray(W1, np.float32)
    As1 = _amat(np.asarray(att_src1, np.float32), FH1, HID, HEADS)
    Ad1 = _amat(np.asarray(att_dst1, np.float32), FH1, HID, HEADS)
    w1ext = _to_bf16(np.concatenate([W1, W1 @ As1, W1 @ Ad1], axis=1))
    b1row = np.tile(np.asarray(b1, np.float32)[None, :], (128, 1))

    in_maps = []
    for c in range(NC):
        xsh = np.zeros((NSH, FIN), np.float32)
        xsh[:OWN] = x[nodes_of_core[c]]
        in_maps.append(dict(
            xt=_to_bf16(xsh.T.copy()),
            wext=w1ext, brow=b1row,
            prow=np.full((1, HEADS), -1e4, np.float32),
            idxa=cores[c]["idx_a"],
            idxb=cores[c]["idx_b"],
            idxc=cores[c]["idx_c"],
        ))
    res1 = run_bass_kernel_spmd(nc1, in_maps, list(range(NC)),
                                trace=TRACE, **RUN_KW)

    x2 = np.zeros((N, FH1), np.float32)
    for c in range(NC):
        x2[nodes_of_core[c]] = res1.results[c]["out"][:OWN]

    # ---- layer 2 inputs
    W2 = np.asarray(W2, np.float32)
    As2 = _amat(np.asarray(att_src2, np.float32), CLS, CLS, 1)
    Ad2 = _amat(np.asarray(att_dst2, np.float32), CLS, CLS, 1)
    w2ext = _to_bf16(np.concatenate([W2, W2 @ As2, W2 @ Ad2], axis=1))
    b2row = np.tile(np.asarray(b2, np.float32)[None, :], (128, 1))

    in_maps2 = []
    for c in range(NC):
        xsh = np.zeros((NSH, FH1), np.float32)
        xsh[:OWN] = x2[nodes_of_core[c]]
        in_maps2.append(dict(
            xt=_to_bf16(xsh.T.copy()),
            wext=w2ext, brow=b2row,
            prow=np.full((1, 1), -1e4, np.float32),
            idxa=in_maps[c]["idxa"],
            idxb=in_maps[c]["idxb"],
            idxc=in_maps[c]["idxc"],
        ))
    res2 = run_bass_kernel_spmd(nc2, in_maps2, list(range(NC)),
                                trace=TRACE, **RUN_KW)

    outf = np.zeros((N, CLS), np.float32)
    for c in range(NC):
        outf[nodes_of_core[c]] = res2.results[c]["out"][:OWN]
    kernel.last_results = (res1, res2)
    return outf


# revision 17
# speedup vs baseline: 1.1318x; 1.0469x over previous
"""GAT (2-layer, PyG-style) on 8 Trainium2 NeuronCores via Bass/Tile.

Strategy (dst-major graph-parallel, v2):
  - Nodes globally sorted by in-degree and striped across cores (rank r ->
    core r%8, slot q=r//8).  Every core's block j (128 dsts) then has an
    near-identical degree profile, so one shared rectangular slot grid is
    tight (pad ~1.27x vs 1.48x before).
  - Table row of node = core*6272 + q; the same permutation orders the
    phase-A matmul (h = x @ [W | W@Asrc | W@Adst]), so per-block adst falls
    out of the same matmul (no separate xperm input / matmuls).
  - dma_gather indices are int16 (<32768), so THREE overlapping table
    windows A=[0,32768) B=[8704,41472) C=[17408,50176) are used; per-dst
    edges are split A/B/C by a greedy prefix/suffix fill, which shrinks the
    forced-window maxima that dominated the 2-window split.
  - Pad slots point at per-core poison rows (q=6271) whose asrc is set to
    -1e4 on device, so exp() underflows to exact 0 and no mask multiply or
    alpha normalization per edge is needed; softmax scale (1/den) is
    applied once per dst after the slot reduction.
  - Gathers are issued per GROUP of consecutive blocks (fewer SWDGE calls).
  - Layer 1 output (elu'd) returns to host, which reassembles/transposes
    and launches layer 2 (same machinery, 1 head, 16 classes).

kernel(**inputs) takes FULL unsharded inputs, returns the FULL [50000, 16]
output.  Host-side numpy does sharding/index prep only; all model math runs
on the NeuronCores.
"""

import os
import sys

import numpy as np

sys.path.insert(0, "/opt/trn_rl_repo")

import concourse.bacc as bacc
import concourse.bass as bass
import concourse.mybir as mybir
import concourse.tile as tile
from concourse.bass_utils import run_bass_kernel_spmd

F32 = mybir.dt.float32
BF16 = mybir.dt.bfloat16
I16 = mybir.dt.int16

N = 50000
NC = 8
FIN = 128
HID = 16
HEADS = 8
FH1 = HEADS * HID        # 128
CLS = 16
NEG = 0.2
NPAD = 50176             # 392 * 128
NSH = NPAD // NC         # 6272 rows per core shard
NSHC = NSH // 128        # 49 chunks per core == BLKS
BLKS = 49
OWNPAD = BLKS * 128      # 6272
OWN = N // NC            # 6250 real dsts per core
WIN = 32768              # int16 index window
OA, OB, OC = 0, 8704, NPAD - WIN          # 0, 8704, 17408
POISON_A = 6271                            # core0 q=6271 (< OB)
POISON_B = 2 * NSH + 6271 - OB             # core2 q=6271, B-window coords
POISON_C = 7 * NSH + 6271 - OC             # core7 q=6271 -> 32767

# Layer table layouts (f32-typed rows; gather moves bytes).
# L1 row (128 f32 = 512B): [h bf16 x128 (f32 cols 0:64) | psum-junk | asrc
# f32 x8 at cols 120:128].
ROW1 = 128
A1OFF = 120
# L2 row (64 f32 = 256B): [h2 bf16 x16 (f32 cols 0:8) | junk | asrc2 at 63]
ROW2 = 64
A2OFF = 63

GCAP1 = 56               # max slots per gather group, layer 1 (512B rows)
GCAP2 = 72               # layer 2 (256B rows)


# ---------------------------------------------------------------- host prep

def _prep(edge_index):
    """Degree-striped node layout + 3-window slot grid. Pure numpy."""
    ei = np.asarray(edge_index)
    loop = np.arange(N, dtype=np.int64)
    src = np.concatenate([ei[0].astype(np.int64), loop])
    dst = np.concatenate([ei[1].astype(np.int64), loop])

    deg_n = np.bincount(dst, minlength=N)
    order = np.argsort(-deg_n, kind="stable")      # rank r -> node
    rank = np.empty(N, np.int64)
    rank[order] = np.arange(N)
    core_n = rank % NC
    q_n = rank // NC                               # < 6250 <= 6271
    row_n = core_n * NSH + q_n                     # table/grid row

    dcore = core_n[dst]
    dq = q_n[dst]
    srow = row_n[src]
    key = dcore * NSH + dq                         # per-(core,dst) id
    M = NC * NSH
    blk_of = (np.arange(M) % NSH) // 128

    degq = np.bincount(key, minlength=M)
    kA = np.bincount(key[srow < OA + WIN], minlength=M)   # A-coverable prefix
    kC = np.bincount(key[srow >= OC], minlength=M)        # C-coverable suffix
    nA = np.bincount(key[srow < OB], minlength=M)         # A-only
    nC = np.bincount(key[srow >= OB + WIN], minlength=M)  # C-only

    def bmax(x):
        return x.reshape(NC, BLKS, 128).max(axis=(0, 2)).astype(np.int64)

    # per-block caps: small search over bumps of the A/C caps to minimize
    # the induced B cap (greedy fill: A takes the sorted prefix, C the
    # suffix, B the middle).
    base_A = np.maximum(bmax(nA), 1)
    base_C = np.maximum(bmax(nC), 1)
    best_T = None
    for ba in range(3):
        for bc in range(3):
            SA = base_A + ba
            SC = base_C + bc
            aA_t = np.minimum(kA, SA[blk_of])
            remC_t = kC - np.maximum(0, aA_t - (degq - kC))
            aC_t = np.minimum(remC_t, SC[blk_of])
            SB = np.maximum(bmax(np.maximum(degq - aA_t - aC_t, 0)), 1)
            ST = SA + SB + SC
            if best_T is None:
                best_T = ST.copy()
                S_A, S_B, S_C = SA.copy(), SB.copy(), SC.copy()
            else:
                better = ST < best_T
                best_T = np.where(better, ST, best_T)
                S_A = np.where(better, SA, S_A)
                S_B = np.where(better, SB, S_B)
                S_C = np.where(better, SC, S_C)
    aA = np.minimum(kA, S_A[blk_of])
    remC = kC - np.maximum(0, aA - (degq - kC))
    aC = np.minimum(remC, S_C[blk_of])
    assert np.all(degq - aA - aC <= S_B[blk_of])
    S_T = S_A + S_B + S_C

    ALOP = np.concatenate([[0], np.cumsum(S_A)]).astype(int)
    BLOP = np.concatenate([[0], np.cumsum(S_B)]).astype(int)
    CLOP = np.concatenate([[0], np.cumsum(S_C)]).astype(int)
    SLA, SLB, SLC = int(ALOP[-1]), int(BLOP[-1]), int(CLOP[-1])

    # per-edge slot assignment: sort by (dst-key, src-row); within dst the
    # first aA go to A, last aC to C, middle to B.
    eorder = np.lexsort((srow, key))
    ks, ss = key[eorder], srow[eorder]
    uniq, first_idx, counts = np.unique(ks, return_index=True,
                                        return_counts=True)
    pos = np.arange(len(ks)) - np.repeat(first_idx, counts)   # 0..deg-1
    deg_e = degq[ks]
    aA_e = aA[ks]
    aC_e = aC[ks]
    in_A = pos < aA_e
    in_C = pos >= (deg_e - aC_e)
    in_B = ~(in_A | in_C)
    blk_e = (ks % NSH) // 128
    p_e = (ks % NSH) % 128
    c_e = ks // NSH
    # feasibility / coverage asserts
    assert np.all(ss[in_A] < OA + WIN)
    assert np.all((ss[in_B] >= OB) & (ss[in_B] < OB + WIN))
    assert np.all(ss[in_C] >= OC)
    colA = ALOP[blk_e] + pos
    colB = BLOP[blk_e] + (pos - aA_e)
    colC = CLOP[blk_e] + (pos - (deg_e - aC_e))
    assert np.all(colA[in_A] < ALOP[blk_e[in_A] + 1])
    assert np.all(colB[in_B] < BLOP[blk_e[in_B] + 1])
    assert np.all(colC[in_C] < CLOP[blk_e[in_C] + 1])

    cores = []
    for c in range(NC):
        m = c_e == c
        idx_a = np.full((SLA, 128), POISON_A, np.int16)    # [col, partition]
        idx_b = np.full((SLB, 128), POISON_B, np.int16)
        idx_c = np.full((SLC, 128), POISON_C, np.int16)
        ma = m & in_A
        idx_a[colA[ma], p_e[ma]] = (ss[ma] - OA).astype(np.int16)
        mb = m & in_B
        idx_b[colB[mb], p_e[mb]] = (ss[mb] - OB).astype(np.int16)
        mc = m & in_C
        idx_c[colC[mc], p_e[mc]] = (ss[mc] - OC).astype(np.int16)
        cores.append(dict(
            idx_a=_wrap_idx(idx_a.T.copy()),
            idx_b=_wrap_idx(idx_b.T.copy()),
            idx_c=_wrap_idx(idx_c.T.copy()),
        ))

    # gather groups: consecutive blocks, slot total capped
    def mkgroups(cap):
        groups = []
        j = 0
        while j < BLKS:
            j2 = j + 1
            while j2 < BLKS and S_T[j:j2 + 1].sum() <= cap:
                j2 += 1
            groups.append((j, j2))
            j = j2
        return groups

    grids = dict(S_A=S_A, S_B=S_B, S_C=S_C, S_T=S_T,
                 ALOP=ALOP, BLOP=BLOP, CLOP=CLOP,
                 SLA=SLA, SLB=SLB, SLC=SLC,
                 groups1=mkgroups(GCAP1), groups2=mkgroups(GCAP2))
    # node ordering for host-side shard/unshard
    nodes_of_core = [order[c::NC] for c in range(NC)]      # index q -> node
    return cores, grids, nodes_of_core


def _wrap_idx(idx_pc):
    """[128, COLS] per-(partition,col) int16 -> dma_gather idx tile layout.

    dma_gather reads idx position i at sbuf [i%16, i//16] (int16), replicated
    across all 8 groups of 16 partitions.  Position i maps to output
    (partition i%128, col i//128).
    """
    P, C = idx_pc.shape
    assert P == 128
    flat = idx_pc.T.reshape(-1)            # position i = p + 128*c
    n16 = (len(flat) + 15) // 16
    t = np.zeros((16, n16), np.int16)
    t[np.arange(len(flat)) % 16, np.arange(len(flat)) // 16] = flat
    return np.tile(t, (8, 1))              # [128, n16]


# ------------------------------------------------------------- bass builder

def _build_layer(grids, layer):
    """One GAT layer as a Bass SPMD program.

    layer 1: FIN=128 in, 8 heads x 16 -> out 128 (elu'd)
    layer 2: 128 in, 1 head x 16 -> out 16 (+bias only)
    """
    S_A, S_B, S_C, S_T = (grids["S_A"], grids["S_B"], grids["S_C"],
                          grids["S_T"])
    ALOP, BLOP, CLOP = grids["ALOP"], grids["BLOP"], grids["CLOP"]
    SLA, SLB, SLC = grids["SLA"], grids["SLB"], grids["SLC"]
    groups = grids["groups1"] if layer == 1 else grids["groups2"]

    if layer == 1:
        FH, AH, ROW, AOFF = FH1, HEADS, ROW1, A1OFF
        WCOLS = FH + 2 * AH      # 144: [W | W@Asrc | W@Adst]
        FOUT = FH1
        CP0, CP1 = 72, 64        # tail copy psum[:, CP0:CP0+64] -> st[:, 64:]
    else:
        FH, AH, ROW, AOFF = CLS, 1, ROW2, A2OFF
        WCOLS = FH + 2 * AH      # 18
        FOUT = CLS

    nc = bacc.Bacc("TRN2", target_bir_lowering=False, debug=False,
                   num_devices=NC)
    xt = nc.declare_dram_parameter("xt", [128, NSH], BF16, isOutput=False)
    wext = nc.declare_dram_parameter("wext", [128, WCOLS], BF16,
                                     isOutput=False)
    brow = nc.declare_dram_parameter("brow", [128, FOUT], F32, isOutput=False)
    idxa = nc.declare_dram_parameter("idxa", [128, 8 * SLA], I16,
                                     isOutput=False)
    idxb = nc.declare_dram_parameter("idxb", [128, 8 * SLB], I16,
                                     isOutput=False)
    idxc = nc.declare_dram_parameter("idxc", [128, 8 * SLC], I16,
                                     isOutput=False)
    prow = nc.declare_dram_parameter("prow", [1, AH], F32, isOutput=False)
    out = nc.declare_dram_parameter("out", [OWNPAD, FOUT], F32, isOutput=True)
    th_sh = nc.dram_tensor("th_sh", [NSH, ROW], F32)
    th = nc.dram_tensor("th", [NPAD, ROW], F32, addr_space="Shared")

    SMAX = int(S_T.max())

    with tile.TileContext(nc) as tc:
        with (
            tc.tile_pool(name="const", bufs=1) as cpool,
            tc.tile_pool(name="psA", bufs=2, space="PSUM") as psA,
            tc.tile_pool(name="ga", bufs=3) as gapool,
            tc.tile_pool(name="gb", bufs=3) as gbpool,
            tc.tile_pool(name="gc", bufs=3) as gcpool,
            tc.tile_pool(name="ep", bufs=2) as epool,
            tc.tile_pool(name="msg", bufs=2) as mpool,
            tc.tile_pool(name="fin", bufs=3) as fpool,
        ):
            # constants
            w_sb = cpool.tile([128, WCOLS], BF16)
            nc.sync.dma_start(w_sb[:], wext[:])
            b_sb = cpool.tile([128, FOUT], F32)
            nc.sync.dma_start(b_sb[:], brow[:])
            ia_sb = cpool.tile([128, 8 * SLA], I16)
            nc.sync.dma_start(ia_sb[:], idxa[:])
            ib_sb = cpool.tile([128, 8 * SLB], I16)
            nc.sync.dma_start(ib_sb[:], idxb[:])
            ic_sb = cpool.tile([128, 8 * SLC], I16)
            nc.sync.dma_start(ic_sb[:], idxc[:])
            adst_all = cpool.tile([128, BLKS * AH], F32)
            xt_sb = cpool.tile([128, NSH], BF16)
            nc.sync.dma_start(xt_sb[:], xt[:])
            st_all = cpool.tile([128, NSHC, ROW], F32)
            if layer == 2:
                nc.vector.memset(st_all[:], 0.0)

            stf = st_all[:].rearrange("p i r -> p (i r)")       # [128, NSHC*ROW]
            stb = st_all.bitcast(BF16)[:].rearrange("p i r -> p (i r)")

            # ---- phase A: th[n] = [h(n) bf16 | junk | asrc(n) f32]
            for i in range(NSHC):
                ph = psA.tile([128, WCOLS], F32)
                nc.tensor.matmul(ph[:], xt_sb[:, i * 128:(i + 1) * 128],
                                 w_sb[:], start=True, stop=True)
                # h -> bf16 (cast on copy); tail cols f32 incl asrc
                nc.scalar.copy(stb[:, i * 2 * ROW:i * 2 * ROW + FH],
                               ph[:, 0:FH])
                if layer == 1:
                    nc.vector.tensor_copy(
                        stf[:, i * ROW + CP1:(i + 1) * ROW],
                        ph[:, CP0:CP0 + (ROW - CP1)])
                else:
                    nc.vector.tensor_copy(
                        stf[:, i * ROW + AOFF:i * ROW + AOFF + 1],
                        ph[:, FH:FH + 1])
                nc.vector.tensor_copy(adst_all[:, i * AH:(i + 1) * AH],
                                      ph[:, FH + AH:FH + 2 * AH])
            nc.sync.dma_start(
                th_sh[:].rearrange("(i p) r -> p i r", p=128), st_all[:])
            # poison row q=6271: asrc -> -1e4 so exp underflows to exact 0.
            # Same sync queue as the store above -> FIFO-ordered after it;
            # the barrier below orders it vs AllGather.
            nc.sync.dma_start(th_sh[NSH - 1:NSH, AOFF:AOFF + AH], prow[:])

            tc.strict_bb_all_engine_barrier()
            nc.gpsimd.collective_compute(
                "AllGather", mybir.AluOpType.bypass,
                replica_groups=[list(range(NC))],
                ins=[th_sh[:]], outs=[th[:]])
            tc.strict_bb_all_engine_barrier()

            # ---- phase B: per gather-group of consecutive 128-dst blocks
            for (j0, j1) in groups:
                SAg = int(ALOP[j1] - ALOP[j0])
                SBg = int(BLOP[j1] - BLOP[j0])
                SCg = int(CLOP[j1] - CLOP[j0])
                ga = gapool.tile([128, SAg, ROW], F32, tag="ga")
                nc.gpsimd.dma_gather(
                    ga[:], th[OA:OA + WIN, :],
                    ia_sb[:, 8 * ALOP[j0]: 8 * (ALOP[j0] + SAg)],
                    num_idxs=128 * SAg, num_idxs_reg=128 * SAg,
                    elem_size=ROW, single_packet=False)
                gb = gbpool.tile([128, SBg, ROW], F32, tag="gb")
                nc.gpsimd.dma_gather(
                    gb[:], th[OB:OB + WIN, :],
                    ib_sb[:, 8 * BLOP[j0]: 8 * (BLOP[j0] + SBg)],
                    num_idxs=128 * SBg, num_idxs_reg=128 * SBg,
                    elem_size=ROW, single_packet=False)
                gc = gcpool.tile([128, SCg, ROW], F32, tag="gc")
                nc.gpsimd.dma_gather(
                    gc[:], th[OC:OC + WIN, :],
                    ic_sb[:, 8 * CLOP[j0]: 8 * (CLOP[j0] + SCg)],
                    num_idxs=128 * SCg, num_idxs_reg=128 * SCg,
                    elem_size=ROW, single_packet=False)

                nblk = j1 - j0
                fing = fpool.tile([128, nblk, FOUT], F32, tag="fing")
                fing_f = fing[:].rearrange("p j f -> p (j f)")
                for j in range(j0, j1):
                    Sa, Sb, Sc = int(S_A[j]), int(S_B[j]), int(S_C[j])
                    S = Sa + Sb + Sc
                    a0 = int(ALOP[j] - ALOP[j0])
                    b0 = int(BLOP[j] - BLOP[j0])
                    c0 = int(CLOP[j] - CLOP[j0])
                    jj = j - j0
                    adst = adst_all[:, j * AH:(j + 1) * AH]

                    # e = asrc + adst per slot (3 window sub-ranges)
                    e = epool.tile([128, S, AH], F32, tag="e")
                    for (gt, off, S0, Ssub) in ((ga, a0, 0, Sa),
                                                (gb, b0, Sa, Sb),
                                                (gc, c0, Sa + Sb, Sc)):
                        if AH == 1:
                            # adst is a per-partition scalar: add on ACT
                            nc.scalar.activation(
                                e[:, S0:S0 + Ssub, :],
                                gt[:, off:off + Ssub, AOFF:AOFF + AH],
                                mybir.ActivationFunctionType.Identity,
                                bias=adst)
                        else:
                            nc.vector.tensor_tensor(
                                e[:, S0:S0 + Ssub, :],
                                gt[:, off:off + Ssub, AOFF:AOFF + AH],
                                adst.unsqueeze(1)
                                    .broadcast_to([128, Ssub, AH]),
                                op=mybir.AluOpType.add)
                    # lrelu: max(NEG*e, e), then exp (ACT engine)
                    e2 = epool.tile([128, S, AH], F32, tag="e2")
                    nc.vector.scalar_tensor_tensor(
                        e2[:], e[:], NEG, e[:],
                        op0=mybir.AluOpType.mult, op1=mybir.AluOpType.max)
                    pt = epool.tile([128, S, AH], F32, tag="p")
                    den = fpool.tile([128, AH], F32, tag="den")
                    if AH == 1:
                        # denominator falls out of the exp on ACT
                        nc.scalar.activation(pt[:], e2[:],
                                             mybir.ActivationFunctionType.Exp,
                                             accum_out=den[:])
                    else:
                        nc.scalar.activation(pt[:], e2[:],
                                             mybir.ActivationFunctionType.Exp)
                        nc.vector.tensor_reduce(den[:],
                                                pt[:].transpose([0, 2, 1]),
                                                axis=mybir.AxisListType.X,
                                                op=mybir.AluOpType.add)
                    rec = fpool.tile([128, AH], F32, tag="rec")
                    nc.vector.reciprocal(rec[:], den[:])

                    # msg = h_gath * p  (contiguous [p, s, fh], bf16)
                    msg = mpool.tile([128, S, FH], BF16, tag="msg")
                    for (gt, off, S0, Ssub) in ((ga, a0, 0, Sa),
                                                (gb, b0, Sa, Sb),
                                                (gc, c0, Sa + Sb, Sc)):
                        hview = gt.bitcast(BF16)[:, off:off + Ssub, 0:FH]
                        hview = hview.rearrange("p s (h c) -> p s h c", c=HID)
                        nc.vector.tensor_tensor(
                            msg[:, S0:S0 + Ssub]
                               .rearrange("p s (h c) -> p s h c", c=HID),
                            hview,
                            pt[:, S0:S0 + Ssub, :].unsqueeze(3)
                              .broadcast_to([128, Ssub, AH, HID]),
                            op=mybir.AluOpType.mult)
                    outun = fpool.tile([128, FOUT], F32, tag="outun")
                    nc.vector.tensor_reduce(outun[:],
                                            msg[:].transpose([0, 2, 1]),
                                            axis=mybir.AxisListType.X,
                                            op=mybir.AluOpType.add)

                    fin = fing_f[:, jj * FOUT:(jj + 1) * FOUT]
                    if layer == 1:
                        # scale by 1/den (per head), +b, elu
                        nc.vector.tensor_tensor(
                            outun[:].rearrange("p (h c) -> p h c", c=HID),
                            outun[:].rearrange("p (h c) -> p h c", c=HID),
                            rec[:].unsqueeze(2).broadcast_to([128, AH, HID]),
                            op=mybir.AluOpType.mult)
                        nc.vector.tensor_tensor(outun[:], outun[:], b_sb[:],
                                                op=mybir.AluOpType.add)
                        # elu(z) = relu(z) + min(exp(z),1) - 1
                        a1 = fpool.tile([128, FOUT], F32, tag="a1")
                        nc.scalar.activation(a1[:], outun[:],
                                             mybir.ActivationFunctionType.Relu)
                        a2 = fpool.tile([128, FOUT], F32, tag="a2")
                        nc.scalar.activation(a2[:], outun[:],
                                             mybir.ActivationFunctionType.Exp)
                        t3 = fpool.tile([128, FOUT], F32, tag="t3")
                        nc.vector.scalar_tensor_tensor(
                            t3[:], a2[:], 1.0, a1[:],
                            op0=mybir.AluOpType.min, op1=mybir.AluOpType.add)
                        nc.scalar.activation(fin, t3[:],
                                             mybir.ActivationFunctionType.Copy,
                                             bias=-1.0)
                    else:
                        # scale by 1/den (scalar per partition) on ACT, +b
                        sc = fpool.tile([128, FOUT], F32, tag="sc")
                        nc.scalar.activation(sc[:], outun[:],
                                             mybir.ActivationFunctionType.Copy,
                                             scale=rec[:])
                        nc.vector.tensor_tensor(fin, sc[:], b_sb[:],
                                                op=mybir.AluOpType.add)
                nc.sync.dma_start(
                    out[j0 * 128:j1 * 128, :]
                    .rearrange("(jj p) f -> p jj f", p=128),
                    fing[:])

    nc.compile()
    return nc


# --------------------------------------------------------------- execution

_CACHE = {}
TRACE = os.environ.get("GAT_TRACE", "0") == "1"
RUN_KW = {}


def _to_bf16(a):
    return np.asarray(a, np.float32).astype(mybir.dt.np(BF16))


def _amat(att, fh, hid, heads):
    """[heads, hid] attention vec -> [fh, heads] block-diag matrix."""
    m = np.zeros((fh, heads), np.float32)
    for h in range(heads):
        m[h * hid:(h + 1) * hid, h] = att[h]
    return m


def kernel(x, edge_index, W1, att_src1, att_dst1, b1, W2, att_src2, att_dst2,
           b2):
    x = np.asarray(x, np.float32)
    ei = np.asarray(edge_index)
    if "prep" not in _CACHE:
        _CACHE["prep"] = _prep(ei)
    cores, grids, nodes_of_core = _CACHE["prep"]

    if "nc1" not in _CACHE:
        _CACHE["nc1"] = _build_layer(grids, 1)
        _CACHE["nc2"] = _build_layer(grids, 2)
    nc1, nc2 = _CACHE["nc1"], _CACHE["nc2"]

    # ---- layer 1 inputs
    W1 = np.asarray(W1, np.float32)
    As1 = _amat(np.asarray(att_src1, np.float32), FH1, HID, HEADS)
    Ad1 = _amat(np.asarray(att_dst1, np.float32), FH1, HID, HEADS)
    w1ext = _to_bf16(np.concatenate([W1, W1 @ As1, W1 @ Ad1], axis=1))
    b1row = np.tile(np.asarray(b1, np.float32)[None, :], (128, 1))

    in_maps = []
    for c in range(NC):
        xsh = np.zeros((NSH, FIN), np.float32)
        xsh[:OWN] = x[nodes_of_core[c]]
        in_maps.append(dict(
            xt=_to_bf16(xsh.T.copy()),
            wext=w1ext, brow=b1row,
            prow=np.full((1, HEADS), -1e4, np.float32),
            idxa=cores[c]["idx_a"],
            idxb=cores[c]["idx_b"],
            idxc=cores[c]["idx_c"],
        ))
    res1 = run_bass_kernel_spmd(nc1, in_maps, list(range(NC)),
                                trace=TRACE, **RUN_KW)

    x2 = np.zeros((N, FH1), np.float32)
    for c in range(NC):
        x2[nodes_of_core[c]] = res1.results[c]["out"][:OWN]

    # ---- layer 2 inputs
    W2 = np.asarray(W2, np.float32)
    As2 = _amat(np.asarray(att_src2, np.float32), CLS, CLS, 1)
    Ad2 = _amat(np.asarray(att_dst2, np.float32), CLS, CLS, 1)
    w2ext = _to_bf16(np.concatenate([W2, W2 @ As2, W2 @ Ad2], axis=1))
    b2row = np.tile(np.asarray(b2, np.float32)[None, :], (128, 1))

    in_maps2 = []
    for c in range(NC):
        xsh = np.zeros((NSH, FH1), np.float32)
        xsh[:OWN] = x2[nodes_of_core[c]]
        in_maps2.append(dict(
            xt=_to_bf16(xsh.T.copy()),
            wext=w2ext, brow=b2row,
            prow=np.full((1, 1), -1e4, np.float32),
            idxa=in_maps[c]["idxa"],
            idxb=in_maps[c]["idxb"],
            idxc=in_maps[c]["idxc"],
        ))
    res2 = run_bass_kernel_spmd(nc2, in_maps2, list(range(NC)),
                                trace=TRACE, **RUN_KW)

    outf = np.zeros((N, CLS), np.float32)
    for c in range(NC):
        outf[nodes_of_core[c]] = res2.results[c]["out"][:OWN]
    kernel.last_results = (res1, res2)
    return outf


# revision 20
# speedup vs baseline: 1.1536x; 1.0193x over previous
"""GAT (2-layer, PyG-style) on 8 Trainium2 NeuronCores via Bass/Tile.

Strategy (dst-major graph-parallel, v3 — both layers fused in one program):
  - Nodes globally sorted by in-degree and striped across cores (rank r ->
    core r%8, slot q=r//8).  Every core's block j (128 dsts) then has a
    near-identical degree profile, so one shared rectangular slot grid is
    tight (pad ~1.27x).
  - Table row of node = core*6272 + q; the same permutation orders the
    phase-A matmul (h = x @ [W | W@Asrc | W@Adst]), so per-block adst falls
    out of the same matmul.
  - dma_gather indices are int16 (<32768), so THREE overlapping table
    windows A=[0,32768) B=[8704,41472) C=[17408,50176) are used; per-dst
    edges are split A/B/C by a greedy prefix/suffix fill.
  - Pad slots point at per-core poison rows (q=6271) whose asrc is set to
    -1e4 on device, so exp() underflows to exact 0: no mask multiply, no
    per-edge alpha normalize (1/den applied once per dst post-reduction).
  - Gathers are issued per GROUP of consecutive blocks, 3 bufs deep.
  - Layers are FUSED: as each layer-1 block finishes, its elu'd output is
    transposed on TensorE and pushed through W2 immediately (hidden behind
    layer-1's gather-bound phase), so only the second AllGather separates
    the two edge-processing phases.

kernel(**inputs) takes FULL unsharded inputs, returns the FULL [50000, 16]
output.  Host-side numpy does sharding/index prep only; all model math runs
on the NeuronCores.
"""

import os
import sys

import numpy as np

sys.path.insert(0, "/opt/trn_rl_repo")

import concourse.bacc as bacc
import concourse.bass as bass
import concourse.mybir as mybir
import concourse.tile as tile
from concourse.bass_utils import run_bass_kernel_spmd

F32 = mybir.dt.float32
BF16 = mybir.dt.bfloat16
I16 = mybir.dt.int16

N = 50000
NC = 8
FIN = 128
HID = 16
HEADS = 8
FH1 = HEADS * HID        # 128
CLS = 16
NEG = 0.2
NPAD = 50176             # 392 * 128
NSH = NPAD // NC         # 6272 rows per core shard
NSHC = NSH // 128        # 49 chunks per core == BLKS
BLKS = 49
OWNPAD = BLKS * 128      # 6272
OWN = N // NC            # 6250 real dsts per core
WIN = 32768              # int16 index window
OA, OB, OC = 0, 8704, NPAD - WIN          # 0, 8704, 17408
POISON_A = 6271                            # core0 q=6271 (< OB)
POISON_B = 2 * NSH + 6271 - OB             # core2 q=6271, B-window coords
POISON_C = 7 * NSH + 6271 - OC             # core7 q=6271 -> 32767

# Layer table layouts (f32-typed rows; gather moves bytes).
ROW1 = 128               # [h bf16 x128 | psum junk | asrc f32 x8 @120]
A1OFF = 120
ROW2 = 64                # [h2 bf16 x16 | junk | asrc2 f32 @63]
A2OFF = 63

GCAP1 = 56               # max slots per gather group, layer 1 (512B rows)
GCAP2 = 72               # layer 2 (256B rows)


# ---------------------------------------------------------------- host prep

def _prep(edge_index):
    """Degree-striped node layout + 3-window slot grid. Pure numpy."""
    ei = np.asarray(edge_index)
    loop = np.arange(N, dtype=np.int64)
    src = np.concatenate([ei[0].astype(np.int64), loop])
    dst = np.concatenate([ei[1].astype(np.int64), loop])

    deg_n = np.bincount(dst, minlength=N)
    order = np.argsort(-deg_n, kind="stable")      # rank r -> node
    rank = np.empty(N, np.int64)
    rank[order] = np.arange(N)
    core_n = rank % NC
    q_n = rank // NC                               # < 6250 <= 6271
    row_n = core_n * NSH + q_n                     # table/grid row

    dcore = core_n[dst]
    dq = q_n[dst]
    srow = row_n[src]
    key = dcore * NSH + dq                         # per-(core,dst) id
    M = NC * NSH
    blk_of = (np.arange(M) % NSH) // 128

    degq = np.bincount(key, minlength=M)
    kA = np.bincount(key[srow < OA + WIN], minlength=M)   # A-coverable prefix
    kC = np.bincount(key[srow >= OC], minlength=M)        # C-coverable suffix
    nA = np.bincount(key[srow < OB], minlength=M)         # A-only
    nC = np.bincount(key[srow >= OB + WIN], minlength=M)  # C-only

    def bmax(x):
        return x.reshape(NC, BLKS, 128).max(axis=(0, 2)).astype(np.int64)

    # per-block caps: small search over bumps of the A/C caps to minimize
    # the induced B cap (greedy fill: A takes the sorted prefix, C the
    # suffix, B the middle).
    base_A = np.maximum(bmax(nA), 1)
    base_C = np.maximum(bmax(nC), 1)
    best_T = None
    for ba in range(3):
        for bc in range(3):
            SA = base_A + ba
            SC = base_C + bc
            aA_t = np.minimum(kA, SA[blk_of])
            remC_t = kC - np.maximum(0, aA_t - (degq - kC))
            aC_t = np.minimum(remC_t, SC[blk_of])
            SB = np.maximum(bmax(np.maximum(degq - aA_t - aC_t, 0)), 1)
            ST = SA + SB + SC
            if best_T is None:
                best_T = ST.copy()
                S_A, S_B, S_C = SA.copy(), SB.copy(), SC.copy()
            else:
                better = ST < best_T
                best_T = np.where(better, ST, best_T)
                S_A = np.where(better, SA, S_A)
                S_B = np.where(better, SB, S_B)
                S_C = np.where(better, SC, S_C)
    aA = np.minimum(kA, S_A[blk_of])
    remC = kC - np.maximum(0, aA - (degq - kC))
    aC = np.minimum(remC, S_C[blk_of])
    assert np.all(degq - aA - aC <= S_B[blk_of])
    S_T = S_A + S_B + S_C

    ALOP = np.concatenate([[0], np.cumsum(S_A)]).astype(int)
    BLOP = np.concatenate([[0], np.cumsum(S_B)]).astype(int)
    CLOP = np.concatenate([[0], np.cumsum(S_C)]).astype(int)
    SLA, SLB, SLC = int(ALOP[-1]), int(BLOP[-1]), int(CLOP[-1])

    # per-edge slot assignment: sort by (dst-key, src-row); within dst the
    # first aA go to A, last aC to C, middle to B.
    eorder = np.lexsort((srow, key))
    ks, ss = key[eorder], srow[eorder]
    uniq, first_idx, counts = np.unique(ks, return_index=True,
                                        return_counts=True)
    pos = np.arange(len(ks)) - np.repeat(first_idx, counts)   # 0..deg-1
    deg_e = degq[ks]
    aA_e = aA[ks]
    aC_e = aC[ks]
    in_A = pos < aA_e
    in_C = pos >= (deg_e - aC_e)
    in_B = ~(in_A | in_C)
    blk_e = (ks % NSH) // 128
    p_e = (ks % NSH) % 128
    c_e = ks // NSH
    # feasibility / coverage asserts
    assert np.all(ss[in_A] < OA + WIN)
    assert np.all((ss[in_B] >= OB) & (ss[in_B] < OB + WIN))
    assert np.all(ss[in_C] >= OC)
    colA = ALOP[blk_e] + pos
    colB = BLOP[blk_e] + (pos - aA_e)
    colC = CLOP[blk_e] + (pos - (deg_e - aC_e))
    assert np.all(colA[in_A] < ALOP[blk_e[in_A] + 1])
    assert np.all(colB[in_B] < BLOP[blk_e[in_B] + 1])
    assert np.all(colC[in_C] < CLOP[blk_e[in_C] + 1])

    cores = []
    for c in range(NC):
        m = c_e == c
        idx_a = np.full((SLA, 128), POISON_A, np.int16)    # [col, partition]
        idx_b = np.full((SLB, 128), POISON_B, np.int16)
        idx_c = np.full((SLC, 128), POISON_C, np.int16)
        ma = m & in_A
        idx_a[colA[ma], p_e[ma]] = (ss[ma] - OA).astype(np.int16)
        mb = m & in_B
        idx_b[colB[mb], p_e[mb]] = (ss[mb] - OB).astype(np.int16)
        mc = m & in_C
        idx_c[colC[mc], p_e[mc]] = (ss[mc] - OC).astype(np.int16)
        cores.append(dict(
            idx_a=_wrap_idx(idx_a.T.copy()),
            idx_b=_wrap_idx(idx_b.T.copy()),
            idx_c=_wrap_idx(idx_c.T.copy()),
        ))

    # gather groups: consecutive blocks, slot total capped
    def mkgroups(cap):
        groups = []
        j = 0
        while j < BLKS:
            j2 = j + 1
            while j2 < BLKS and S_T[j:j2 + 1].sum() <= cap:
                j2 += 1
            groups.append((j, j2))
            j = j2
        return groups

    grids = dict(S_A=S_A, S_B=S_B, S_C=S_C, S_T=S_T,
                 ALOP=ALOP, BLOP=BLOP, CLOP=CLOP,
                 SLA=SLA, SLB=SLB, SLC=SLC,
                 groups1=mkgroups(GCAP1), groups2=mkgroups(GCAP2))
    # node ordering for host-side shard/unshard
    nodes_of_core = [order[c::NC] for c in range(NC)]      # index q -> node
    return cores, grids, nodes_of_core


def _wrap_idx(idx_pc):
    """[128, COLS] per-(partition,col) int16 -> dma_gather idx tile layout.

    dma_gather reads idx position i at sbuf [i%16, i//16] (int16), replicated
    across all 8 groups of 16 partitions.  Position i maps to output
    (partition i%128, col i//128).
    """
    P, C = idx_pc.shape
    assert P == 128
    flat = idx_pc.T.reshape(-1)            # position i = p + 128*c
    n16 = (len(flat) + 15) // 16
    t = np.zeros((16, n16), np.int16)
    t[np.arange(len(flat)) % 16, np.arange(len(flat)) // 16] = flat
    return np.tile(t, (8, 1))              # [128, n16]


# ------------------------------------------------------------- bass builder

def _build_fused(grids):
    """Both GAT layers as one Bass SPMD program."""
    S_A, S_B, S_C = grids["S_A"], grids["S_B"], grids["S_C"]
    ALOP, BLOP, CLOP = grids["ALOP"], grids["BLOP"], grids["CLOP"]
    SLA, SLB, SLC = grids["SLA"], grids["SLB"], grids["SLC"]

    W1COLS = FH1 + 2 * HEADS     # 144
    W2COLS = CLS + 2             # 18

    nc = bacc.Bacc("TRN2", target_bir_lowering=False, debug=False,
                   num_devices=NC)
    xt = nc.declare_dram_parameter("xt", [128, NSH], BF16, isOutput=False)
    w1ext = nc.declare_dram_parameter("w1ext", [128, W1COLS], BF16,
                                      isOutput=False)
    w2ext = nc.declare_dram_parameter("w2ext", [128, W2COLS], BF16,
                                      isOutput=False)
    b1row = nc.declare_dram_parameter("b1row", [128, FH1], F32,
                                      isOutput=False)
    b2row = nc.declare_dram_parameter("b2row", [128, CLS], F32,
                                      isOutput=False)
    ident = nc.declare_dram_parameter("ident", [128, 128], BF16,
                                      isOutput=False)
    idxa = nc.declare_dram_parameter("idxa", [128, 8 * SLA], I16,
                                     isOutput=False)
    idxb = nc.declare_dram_parameter("idxb", [128, 8 * SLB], I16,
                                     isOutput=False)
    idxc = nc.declare_dram_parameter("idxc", [128, 8 * SLC], I16,
                                     isOutput=False)
    prow1 = nc.declare_dram_parameter("prow1", [1, HEADS], F32,
                                      isOutput=False)
    prow2 = nc.declare_dram_parameter("prow2", [1, 1], F32, isOutput=False)
    out = nc.declare_dram_parameter("out", [OWNPAD, CLS], F32, isOutput=True)
    th1_sh = nc.dram_tensor("th1_sh", [NSH, ROW1], F32)
    th1 = nc.dram_tensor("th1", [NPAD, ROW1], F32, addr_space="Shared")
    th2_sh = nc.dram_tensor("th2_sh", [NSH, ROW2], F32)
    th2 = nc.dram_tensor("th2", [NPAD, ROW2], F32, addr_space="Shared")

    with tile.TileContext(nc) as tc:
        with (
            tc.tile_pool(name="const", bufs=1) as cpool,
            tc.tile_pool(name="psA", bufs=2, space="PSUM") as psA,
            tc.tile_pool(name="psT", bufs=2, space="PSUM") as psT,
        ):
            w1_sb = cpool.tile([128, W1COLS], BF16)
            nc.sync.dma_start(w1_sb[:], w1ext[:])
            w2_sb = cpool.tile([128, W2COLS], BF16)
            nc.sync.dma_start(w2_sb[:], w2ext[:])
            b1_sb = cpool.tile([128, FH1], F32)
            nc.sync.dma_start(b1_sb[:], b1row[:])
            b2_sb = cpool.tile([128, CLS], F32)
            nc.sync.dma_start(b2_sb[:], b2row[:])
            id_sb = cpool.tile([128, 128], BF16)
            nc.sync.dma_start(id_sb[:], ident[:])
            ia_sb = cpool.tile([128, 8 * SLA], I16)
            nc.sync.dma_start(ia_sb[:], idxa[:])
            ib_sb = cpool.tile([128, 8 * SLB], I16)
            nc.sync.dma_start(ib_sb[:], idxb[:])
            ic_sb = cpool.tile([128, 8 * SLC], I16)
            nc.sync.dma_start(ic_sb[:], idxc[:])
            adst1_all = cpool.tile([128, BLKS * HEADS], F32)
            adst2_all = cpool.tile([128, BLKS], F32)
            x2t_sb = cpool.tile([128, NSH], BF16)
            st2_all = cpool.tile([128, NSHC, ROW2], F32)
            nc.vector.memset(st2_all[:], 0.0)
            st2f = st2_all[:].rearrange("p i r -> p (i r)")
            st2b = st2_all.bitcast(BF16)[:].rearrange("p i r -> p (i r)")

            # ---- phase A (layer 1): th1[n] = [h bf16 | junk | asrc f32]
            with tc.tile_pool(name="pa", bufs=1) as papool:
                xt_sb = papool.tile([128, NSH], BF16)
                nc.sync.dma_start(xt_sb[:], xt[:])
                st1_all = papool.tile([128, NSHC, ROW1], F32)
                st1f = st1_all[:].rearrange("p i r -> p (i r)")
                st1b = st1_all.bitcast(BF16)[:].rearrange("p i r -> p (i r)")
                for i in range(NSHC):
                    ph = psA.tile([128, W1COLS], F32)
                    nc.tensor.matmul(ph[:], xt_sb[:, i * 128:(i + 1) * 128],
                                     w1_sb[:], start=True, stop=True)
                    nc.scalar.copy(st1b[:, i * 2 * ROW1:i * 2 * ROW1 + FH1],
                                   ph[:, 0:FH1])
                    nc.vector.tensor_copy(st1f[:, i * ROW1 + 64:(i + 1) * ROW1],
                                          ph[:, 72:136])
                    nc.vector.tensor_copy(
                        adst1_all[:, i * HEADS:(i + 1) * HEADS],
                        ph[:, FH1 + HEADS:FH1 + 2 * HEADS])
                nc.sync.dma_start(
                    th1_sh[:].rearrange("(i p) r -> p i r", p=128),
                    st1_all[:])
                nc.sync.dma_start(th1_sh[NSH - 1:NSH, A1OFF:A1OFF + HEADS],
                                  prow1[:])

            tc.strict_bb_all_engine_barrier()
            nc.gpsimd.collective_compute(
                "AllGather", mybir.AluOpType.bypass,
                replica_groups=[list(range(NC))],
                ins=[th1_sh[:]], outs=[th1[:]])
            tc.strict_bb_all_engine_barrier()

            # ---- phase B (layer 1) with layer-2 table build interleaved
            with (
                tc.tile_pool(name="ga1", bufs=3) as gapool,
                tc.tile_pool(name="gb1", bufs=3) as gbpool,
                tc.tile_pool(name="gc1", bufs=3) as gcpool,
                tc.tile_pool(name="ep1", bufs=3) as epool,
                tc.tile_pool(name="msg1", bufs=3) as mpool,
                tc.tile_pool(name="fin1", bufs=4) as fpool,
            ):
                for (j0, j1) in grids["groups1"]:
                    SAg = int(ALOP[j1] - ALOP[j0])
                    SBg = int(BLOP[j1] - BLOP[j0])
                    SCg = int(CLOP[j1] - CLOP[j0])
                    ga = gapool.tile([128, SAg, ROW1], F32, tag="ga")
                    nc.gpsimd.dma_gather(
                        ga[:], th1[OA:OA + WIN, :],
                        ia_sb[:, 8 * ALOP[j0]: 8 * (ALOP[j0] + SAg)],
                        num_idxs=128 * SAg, num_idxs_reg=128 * SAg,
                        elem_size=ROW1, single_packet=False)
                    gb = gbpool.tile([128, SBg, ROW1], F32, tag="gb")
                    nc.gpsimd.dma_gather(
                        gb[:], th1[OB:OB + WIN, :],
                        ib_sb[:, 8 * BLOP[j0]: 8 * (BLOP[j0] + SBg)],
                        num_idxs=128 * SBg, num_idxs_reg=128 * SBg,
                        elem_size=ROW1, single_packet=False)
                    gc = gcpool.tile([128, SCg, ROW1], F32, tag="gc")
                    nc.gpsimd.dma_gather(
                        gc[:], th1[OC:OC + WIN, :],
                        ic_sb[:, 8 * CLOP[j0]: 8 * (CLOP[j0] + SCg)],
                        num_idxs=128 * SCg, num_idxs_reg=128 * SCg,
                        elem_size=ROW1, single_packet=False)

                    for j in range(j0, j1):
                        Sa, Sb, Sc = int(S_A[j]), int(S_B[j]), int(S_C[j])
                        S = Sa + Sb + Sc
                        a0 = int(ALOP[j] - ALOP[j0])
                        b0 = int(BLOP[j] - BLOP[j0])
                        c0 = int(CLOP[j] - CLOP[j0])
                        adst = adst1_all[:, j * HEADS:(j + 1) * HEADS]

                        e = epool.tile([128, S, HEADS], F32, tag="e")
                        for (gt, off, S0, Ssub) in ((ga, a0, 0, Sa),
                                                    (gb, b0, Sa, Sb),
                                                    (gc, c0, Sa + Sb, Sc)):
                            nc.vector.tensor_tensor(
                                e[:, S0:S0 + Ssub, :],
                                gt[:, off:off + Ssub, A1OFF:A1OFF + HEADS],
                                adst.unsqueeze(1)
                                    .broadcast_to([128, Ssub, HEADS]),
                                op=mybir.AluOpType.add)
                        e2 = epool.tile([128, S, HEADS], F32, tag="e2")
                        nc.vector.scalar_tensor_tensor(
                            e2[:], e[:], NEG, e[:],
                            op0=mybir.AluOpType.mult,
                            op1=mybir.AluOpType.max)
                        pt = epool.tile([128, S, HEADS], F32, tag="p")
                        nc.scalar.activation(pt[:], e2[:],
                                             mybir.ActivationFunctionType.Exp)
                        den = fpool.tile([128, HEADS], F32, tag="den")
                        nc.vector.tensor_reduce(den[:],
                                                pt[:].transpose([0, 2, 1]),
                                                axis=mybir.AxisListType.X,
                                                op=mybir.AluOpType.add)
                        nc.vector.tensor_scalar_add(den[:], den[:], 1e-16)
                        rec = fpool.tile([128, HEADS], F32, tag="rec")
                        nc.vector.reciprocal(rec[:], den[:])

                        msg = mpool.tile([128, S, FH1], BF16, tag="msg")
                        for (gt, off, S0, Ssub) in ((ga, a0, 0, Sa),
                                                    (gb, b0, Sa, Sb),
                                                    (gc, c0, Sa + Sb, Sc)):
                            hview = gt.bitcast(BF16)[:, off:off + Ssub,
                                                     0:FH1]
                            hview = hview.rearrange("p s (h c) -> p s h c",
                                                    c=HID)
                            nc.vector.tensor_tensor(
                                msg[:, S0:S0 + Ssub]
                                   .rearrange("p s (h c) -> p s h c", c=HID),
                                hview,
                                pt[:, S0:S0 + Ssub, :].unsqueeze(3)
                                  .broadcast_to([128, Ssub, HEADS, HID]),
                                op=mybir.AluOpType.mult)
                        outun = fpool.tile([128, FH1], F32, tag="outun")
                        nc.vector.tensor_reduce(outun[:],
                                                msg[:].transpose([0, 2, 1]),
                                                axis=mybir.AxisListType.X,
                                                op=mybir.AluOpType.add)

                        nc.vector.tensor_tensor(
                            outun[:].rearrange("p (h c) -> p h c", c=HID),
                            outun[:].rearrange("p (h c) -> p h c", c=HID),
                            rec[:].unsqueeze(2)
                                  .broadcast_to([128, HEADS, HID]),
                            op=mybir.AluOpType.mult)
                        nc.vector.tensor_tensor(outun[:], outun[:], b1_sb[:],
                                                op=mybir.AluOpType.add)
                        # elu(z) = relu(z) + min(exp(z),1) - 1
                        a1 = fpool.tile([128, FH1], F32, tag="a1")
                        nc.scalar.activation(a1[:], outun[:],
                                             mybir.ActivationFunctionType.Relu)
                        a2 = fpool.tile([128, FH1], F32, tag="a2")
                        nc.scalar.activation(a2[:], outun[:],
                                             mybir.ActivationFunctionType.Exp)
                        t3 = fpool.tile([128, FH1], F32, tag="t3")
                        nc.vector.scalar_tensor_tensor(
                            t3[:], a2[:], 1.0, a1[:],
                            op0=mybir.AluOpType.min, op1=mybir.AluOpType.add)
                        fin_bf = fpool.tile([128, FH1], BF16, tag="finb")
                        nc.scalar.activation(fin_bf[:], t3[:],
                                             mybir.ActivationFunctionType.Copy,
                                             bias=-1.0)

                        # ---- layer-2 table build for this block (TensorE/
                        # ACT are idle during the gather-bound phase)
                        ptr = psT.tile([128, 128], BF16, tag="ptr")
                        nc.tensor.transpose(ptr[:], fin_bf[:], id_sb[:])
                        nc.scalar.copy(x2t_sb[:, j * 128:(j + 1) * 128],
                                       ptr[:])
                        ph2 = psA.tile([128, W2COLS], F32, tag="ph2")
                        nc.tensor.matmul(ph2[:],
                                         x2t_sb[:, j * 128:(j + 1) * 128],
                                         w2_sb[:], start=True, stop=True)
                        nc.scalar.copy(st2b[:, j * 2 * ROW2:j * 2 * ROW2 + CLS],
                                       ph2[:, 0:CLS])
                        nc.vector.tensor_copy(
                            st2f[:, j * ROW2 + A2OFF:j * ROW2 + A2OFF + 1],
                            ph2[:, CLS:CLS + 1])
                        nc.vector.tensor_copy(adst2_all[:, j:j + 1],
                                              ph2[:, CLS + 1:CLS + 2])

            nc.sync.dma_start(
                th2_sh[:].rearrange("(i p) r -> p i r", p=128), st2_all[:])
            nc.sync.dma_start(th2_sh[NSH - 1:NSH, A2OFF:A2OFF + 1],
                              prow2[:])

            tc.strict_bb_all_engine_barrier()
            nc.gpsimd.collective_compute(
                "AllGather", mybir.AluOpType.bypass,
                replica_groups=[list(range(NC))],
                ins=[th2_sh[:]], outs=[th2[:]])
            tc.strict_bb_all_engine_barrier()

            # ---- phase B (layer 2)
            with (
                tc.tile_pool(name="ga2", bufs=3) as gapool,
                tc.tile_pool(name="gb2", bufs=3) as gbpool,
                tc.tile_pool(name="gc2", bufs=3) as gcpool,
                tc.tile_pool(name="ep2", bufs=2) as epool,
                tc.tile_pool(name="msg2", bufs=2) as mpool,
                tc.tile_pool(name="fin2", bufs=3) as fpool,
            ):
                for (j0, j1) in grids["groups2"]:
                    SAg = int(ALOP[j1] - ALOP[j0])
                    SBg = int(BLOP[j1] - BLOP[j0])
                    SCg = int(CLOP[j1] - CLOP[j0])
                    ga = gapool.tile([128, SAg, ROW2], F32, tag="ga")
                    nc.gpsimd.dma_gather(
                        ga[:], th2[OA:OA + WIN, :],
                        ia_sb[:, 8 * ALOP[j0]: 8 * (ALOP[j0] + SAg)],
                        num_idxs=128 * SAg, num_idxs_reg=128 * SAg,
                        elem_size=ROW2, single_packet=False)
                    gb = gbpool.tile([128, SBg, ROW2], F32, tag="gb")
                    nc.gpsimd.dma_gather(
                        gb[:], th2[OB:OB + WIN, :],
                        ib_sb[:, 8 * BLOP[j0]: 8 * (BLOP[j0] + SBg)],
                        num_idxs=128 * SBg, num_idxs_reg=128 * SBg,
                        elem_size=ROW2, single_packet=False)
                    gc = gcpool.tile([128, SCg, ROW2], F32, tag="gc")
                    nc.gpsimd.dma_gather(
                        gc[:], th2[OC:OC + WIN, :],
                        ic_sb[:, 8 * CLOP[j0]: 8 * (CLOP[j0] + SCg)],
                        num_idxs=128 * SCg, num_idxs_reg=128 * SCg,
                        elem_size=ROW2, single_packet=False)

                    nblk = j1 - j0
                    fing = fpool.tile([128, nblk, CLS], F32, tag="fing")
                    fing_f = fing[:].rearrange("p j f -> p (j f)")
                    for j in range(j0, j1):
                        Sa, Sb, Sc = int(S_A[j]), int(S_B[j]), int(S_C[j])
                        S = Sa + Sb + Sc
                        a0 = int(ALOP[j] - ALOP[j0])
                        b0 = int(BLOP[j] - BLOP[j0])
                        c0 = int(CLOP[j] - CLOP[j0])
                        jj = j - j0
                        adst = adst2_all[:, j:j + 1]

                        e = epool.tile([128, S, 1], F32, tag="e")
                        for (gt, off, S0, Ssub) in ((ga, a0, 0, Sa),
                                                    (gb, b0, Sa, Sb),
                                                    (gc, c0, Sa + Sb, Sc)):
                            nc.scalar.activation(
                                e[:, S0:S0 + Ssub, :],
                                gt[:, off:off + Ssub, A2OFF:A2OFF + 1],
                                mybir.ActivationFunctionType.Identity,
                                bias=adst)
                        e2 = epool.tile([128, S, 1], F32, tag="e2")
                        nc.vector.scalar_tensor_tensor(
                            e2[:], e[:], NEG, e[:],
                            op0=mybir.AluOpType.mult,
                            op1=mybir.AluOpType.max)
                        pt = epool.tile([128, S, 1], F32, tag="p")
                        den = fpool.tile([128, 1], F32, tag="den")
                        nc.scalar.activation(pt[:], e2[:],
                                             mybir.ActivationFunctionType.Exp,
                                             accum_out=den[:])
                        rec = fpool.tile([128, 1], F32, tag="rec")
                        nc.vector.reciprocal(rec[:], den[:])

                        msg = mpool.tile([128, S, CLS], BF16, tag="msg")
                        for (gt, off, S0, Ssub) in ((ga, a0, 0, Sa),
                                                    (gb, b0, Sa, Sb),
                                                    (gc, c0, Sa + Sb, Sc)):
                            hview = gt.bitcast(BF16)[:, off:off + Ssub,
                                                     0:CLS]
                            hview = hview.rearrange("p s (h c) -> p s h c",
                                                    c=HID)
                            nc.vector.tensor_tensor(
                                msg[:, S0:S0 + Ssub]
                                   .rearrange("p s (h c) -> p s h c", c=HID),
                                hview,
                                pt[:, S0:S0 + Ssub, :].unsqueeze(3)
                                  .broadcast_to([128, Ssub, 1, HID]),
                                op=mybir.AluOpType.mult)
                        outun = fpool.tile([128, CLS], F32, tag="outun")
                        nc.vector.tensor_reduce(outun[:],
                                                msg[:].transpose([0, 2, 1]),
                                                axis=mybir.AxisListType.X,
                                                op=mybir.AluOpType.add)

                        sc = fpool.tile([128, CLS], F32, tag="sc")
                        nc.scalar.activation(sc[:], outun[:],
                                             mybir.ActivationFunctionType.Copy,
                                             scale=rec[:])
                        nc.vector.tensor_tensor(
                            fing_f[:, jj * CLS:(jj + 1) * CLS], sc[:],
                            b2_sb[:], op=mybir.AluOpType.add)
                    nc.sync.dma_start(
                        out[j0 * 128:j1 * 128, :]
                        .rearrange("(jj p) f -> p jj f", p=128),
                        fing[:])

    nc.compile()
    return nc


# --------------------------------------------------------------- execution

_CACHE = {}
TRACE = os.environ.get("GAT_TRACE", "0") == "1"
RUN_KW = {}


def _to_bf16(a):
    return np.asarray(a, np.float32).astype(mybir.dt.np(BF16))


def _amat(att, fh, hid, heads):
    """[heads, hid] attention vec -> [fh, heads] block-diag matrix."""
    m = np.zeros((fh, heads), np.float32)
    for h in range(heads):
        m[h * hid:(h + 1) * hid, h] = att[h]
    return m


def kernel(x, edge_index, W1, att_src1, att_dst1, b1, W2, att_src2, att_dst2,
           b2):
    x = np.asarray(x, np.float32)
    ei = np.asarray(edge_index)
    if "prep" not in _CACHE:
        _CACHE["prep"] = _prep(ei)
    cores, grids, nodes_of_core = _CACHE["prep"]

    if "ncf" not in _CACHE:
        _CACHE["ncf"] = _build_fused(grids)
    ncf = _CACHE["ncf"]

    W1 = np.asarray(W1, np.float32)
    As1 = _amat(np.asarray(att_src1, np.float32), FH1, HID, HEADS)
    Ad1 = _amat(np.asarray(att_dst1, np.float32), FH1, HID, HEADS)
    w1e = _to_bf16(np.concatenate([W1, W1 @ As1, W1 @ Ad1], axis=1))
    W2 = np.asarray(W2, np.float32)
    As2 = _amat(np.asarray(att_src2, np.float32), CLS, CLS, 1)
    Ad2 = _amat(np.asarray(att_dst2, np.float32), CLS, CLS, 1)
    w2e = _to_bf16(np.concatenate([W2, W2 @ As2, W2 @ Ad2], axis=1))

    in_maps = []
    for c in range(NC):
        xsh = np.zeros((NSH, FIN), np.float32)
        xsh[:OWN] = x[nodes_of_core[c]]
        in_maps.append(dict(
            xt=_to_bf16(xsh.T.copy()),
            w1ext=w1e, w2ext=w2e,
            b1row=np.tile(np.asarray(b1, np.float32)[None, :], (128, 1)),
            b2row=np.tile(np.asarray(b2, np.float32)[None, :], (128, 1)),
            ident=_to_bf16(np.eye(128, dtype=np.float32)),
            prow1=np.full((1, HEADS), -1e4, np.float32),
            prow2=np.full((1, 1), -1e4, np.float32),
            idxa=cores[c]["idx_a"],
            idxb=cores[c]["idx_b"],
            idxc=cores[c]["idx_c"],
        ))
    res = run_bass_kernel_spmd(ncf, in_maps, list(range(NC)),
                               trace=TRACE, **RUN_KW)

    outf = np.zeros((N, CLS), np.float32)
    for c in range(NC):
        outf[nodes_of_core[c]] = res.results[c]["out"][:OWN]
    kernel.last_results = (res,)
    return outf
